# revision 6
# baseline (speedup 1.0000x reference)
"""CrossFeatureAttention TRN2 kernel (fp8 DoubleRow attention, software-pipelined).

Full inputs -> full output. Sharding: data-parallel over (batch b, half of N1)
across 8 cores; each core computes out[b, h*2048:(h+1)*2048, :].

Math (per core, x1 slice q=2048 rows, x2[b] k=4096 rows, C=512):
    Q  = x1 @ Wq^T + bq                      (bf16 matmul, kept in fp32)
    K  = x2 @ Wk^T          (bk dropped: per-q constant in scores -> softmax inv)
    V  = x2 @ Wv^T          (bv folded into bo2 = Wo bv + bo: softmax rows sum 1)
    P  = exp(Q K^T / sqrt(C))                (S^T via fp8 DR; exp -> fp8)
    rs = colsum(P^T)  (ones fp8 DR matmul)
    A^T = V^T P^T / rs                       (fp8 DR)
    out = (Q + A) @ Wo^T + bo2               (bf16, residual folded via qt reuse)

All attention-path operands are fp8e4 packed in DoubleRow pair layout
[128, 2, N] (two 128-deep contraction planes per matmul -> 2x PE rate).
Host supplies x1^T (bf16) and x2^T / weights (fp8) pre-packed in pair layout
so no on-device transposes and one DMA per tile.

Pipelining: chunk 0's S/exp tiles are interleaved into the KV loop; chunk qc's
A phase is fused with chunk qc+1's S phase so the scalar exp stream hides
under PE work. psum->fp8 copies split across scalar and vector. Startup DMAs
are issued from both HWDGE queues (sync + scalar) to halve issue latency.
"""

import os
import sys

import numpy as np

for _p in ("/root/.axon_site", "/root/.axon_site/_ro/trn_rl_repo",
           "/root/.axon_site/_ro/pypackages"):
    if _p not in sys.path and os.path.isdir(_p):
        sys.path.append(_p)

import ml_dtypes

import concourse.bacc as bacc
import concourse.mybir as mybir
import concourse.tile as tile
from concourse.bass_utils import run_bass_kernel_spmd

F32 = mybir.dt.float32
BF16 = mybir.dt.bfloat16
F8 = mybir.dt.float8e4
AF = mybir.ActivationFunctionType
DR = mybir.MatmulPerfMode.DoubleRow

B, N1, N2, C = 4, 4096, 4096, 512
NCORES = 8
QROWS = N1 * B // NCORES          # 2048 q rows per core
QC = 512                          # q-chunk (columns of S^T tiles)
NQC = QROWS // QC                 # 4 chunks
KT = N2 // 128                    # 32 k-tiles
CCH = C // 128                    # 4 contraction planes of 128
NKP = KT // 2                     # 16 k-plane pairs
SCALE = 1.0 / float(np.sqrt(C))

_BUILT = None


def build():
    nc = bacc.Bacc(None, target_bir_lowering=False, debug=False)

    x1t_d = nc.dram_tensor("x1t", [C, QROWS], BF16, kind="ExternalInput")
    x2p_d = nc.dram_tensor("x2t_p", [256, 2, N2], F8, kind="ExternalInput")
    wq_d = nc.dram_tensor("wq_t", [C, C], BF16, kind="ExternalInput")
    wk_d = nc.dram_tensor("wk_p", [256, 2, C], F8, kind="ExternalInput")
    wv_d = nc.dram_tensor("wv_p", [256, 2, C], F8, kind="ExternalInput")
    wo_d = nc.dram_tensor("wo_t", [C, C], BF16, kind="ExternalInput")
    bq_d = nc.dram_tensor("bq_p", [128, CCH], F32, kind="ExternalInput")
    bo2_d = nc.dram_tensor("bo2", [C], F32, kind="ExternalInput")
    out_d = nc.dram_tensor("out", [QROWS, C], F32, kind="ExternalOutput")

    with tile.TileContext(nc) as tc:
        with tc.tile_pool(name="cst", bufs=1) as cst, \
             tc.tile_pool(name="per", bufs=1) as per, \
             tc.tile_pool(name="sb", bufs=1) as sb, \
             tc.tile_pool(name="ps", bufs=1, space="PSUM") as ps:

            # ---- startup loads, dual-queue: scalar gets the Q-path deps ----
            x1_full = []
            for cc in range(CCH):
                t = per.tile([128, QROWS], BF16, name=f"x1f{cc}", tag=f"x1f{cc}")
                nc.scalar.dma_start(out=t[:], in_=x1t_d[cc * 128:(cc + 1) * 128, :])
                x1_full.append(t)
            wq_b = []
            for cc in range(CCH):
                t = cst.tile([128, C], BF16, name=f"wq{cc}", tag=f"wq{cc}")
                nc.scalar.dma_start(out=t[:], in_=wq_d[cc * 128:(cc + 1) * 128, :])
                wq_b.append(t)
            bq_p = cst.tile([128, CCH], F32)
            nc.scalar.dma_start(out=bq_p[:], in_=bq_d[:])
            bq_t = [bq_p[:, d:d + 1] for d in range(CCH)]

            ones_f8 = cst.tile([128, 2, 128], F8)
            nc.gpsimd.memset(ones_f8[:], 1.0)

            def load_pair_f8(dram, nm):
                ts = []
                for j in range(2):
                    t = cst.tile([128, 2, C], F8, name=f"{nm}{j}", tag=f"{nm}{j}")
                    nc.sync.dma_start(out=t[:], in_=dram[j * 128:(j + 1) * 128, :, :])
                    ts.append(t)
                return ts

            wk_pair = load_pair_f8(wk_d, "wk")
            wv_pair = load_pair_f8(wv_d, "wv")

            # ---- persistent tensors ----
            x2t_pair = [per.tile([128, 2, N2], F8, name=f"x2t{j}", tag=f"x2t{j}")
                        for j in range(2)]
            kt_pair = [per.tile([128, 2, N2], F8, name=f"ktp{j}", tag=f"ktp{j}")
                       for j in range(2)]
            v_pair = [per.tile([128, 2, C], F8, name=f"vp{j}", tag=f"vp{j}")
                      for j in range(NKP)]

            # ---- Q^T projection for one chunk (PE + vector f32 + scalar fp8) ----
            def emit_qproj(qc):
                q0 = qc * QC
                qt_f32 = []
                qt_f8 = [sb.tile([128, 2, QC], F8, name=f"qt8_{j}", tag=f"qt8_{j}", bufs=2)
                         for j in range(2)]
                for dd in range(CCH):
                    pp = ps.tile([128, QC], F32, name="qps", tag="pB", bufs=3)
                    for cc in range(CCH):
                        nc.tensor.matmul(pp[:], lhsT=wq_b[cc][:, dd * 128:(dd + 1) * 128],
                                         rhs=x1_full[cc][:, q0:q0 + QC],
                                         start=(cc == 0), stop=(cc == CCH - 1))
                    t = sb.tile([128, QC], F32, name=f"qtf{dd}", tag=f"qtf{dd}", bufs=2)
                    nc.vector.tensor_add(out=t[:], in0=pp[:],
                                         in1=bq_t[dd].broadcast_to([128, QC]))
                    qt_f32.append(t)
                    nc.scalar.activation(qt_f8[dd // 2][:, dd % 2, :], pp[:],
                                         AF.Identity, bias=bq_t[dd])
                return qt_f32, qt_f8

            def alloc_pt():
                return [sb.tile([128, 2, QC], F8, name=f"pt{j}", tag=f"pt{j}", bufs=2)
                        for j in range(NKP)]

            def emit_s_tile(kt, qt_f8, pt_pair):
                pp = ps.tile([128, QC], F32, name="sps", tag="pA", bufs=3)
                for j in range(2):
                    nc.tensor.matmul(pp[:],
                                     lhsT=kt_pair[j][:, :, kt * 128:(kt + 1) * 128],
                                     rhs=qt_f8[j][:],
                                     start=(j == 0), stop=(j == 1),
                                     perf_mode=DR)
                nc.scalar.activation(pt_pair[kt // 2][:, kt % 2, :], pp[:],
                                     AF.Exp, scale=float(SCALE))

            qt_cur = emit_qproj(0)
            pt_cur = alloc_pt()

            # ---- phase KV (+ chunk-0 S/exp interleaved) ----
            for kc0 in range(N2 // 512):
                if kc0 % 2 == 0:
                    bsl = slice(kc0 * 512, (kc0 + 2) * 512)
                    for j in range(2):
                        nc.sync.dma_start(out=x2t_pair[j][:, :, bsl],
                                          in_=x2p_d[j * 128:(j + 1) * 128, :, bsl])
                ksl = slice(kc0 * 512, (kc0 + 1) * 512)
                # K^T[dd-plane, k-block]; bk dropped (softmax-invariant)
                for dd in range(CCH):
                    pp = ps.tile([128, 512], F32, name="kps", tag="pB", bufs=3)
                    for j in range(2):
                        nc.tensor.matmul(pp[:],
                                         lhsT=wk_pair[j][:, :, dd * 128:(dd + 1) * 128],
                                         rhs=x2t_pair[j][:, :, ksl],
                                         start=(j == 0), stop=(j == 1),
                                         perf_mode=DR)
                    if dd < 2:
                        nc.scalar.copy(kt_pair[dd // 2][:, dd % 2, ksl], pp[:])
                    else:
                        nc.vector.tensor_copy(out=kt_pair[dd // 2][:, dd % 2, ksl],
                                              in_=pp[:])
                # V[k-subtile, :]; bv folded into bo2
                for kb in range(4):
                    ki = kc0 * 4 + kb
                    pp = ps.tile([128, C], F32, name="vps", tag="pB", bufs=3)
                    for j in range(2):
                        nc.tensor.matmul(pp[:],
                                         lhsT=x2t_pair[j][:, :, ki * 128:(ki + 1) * 128],
                                         rhs=wv_pair[j][:],
                                         start=(j == 0), stop=(j == 1),
                                         perf_mode=DR)
                    nc.vector.tensor_copy(out=v_pair[ki // 2][:, ki % 2, :], in_=pp[:])
                # chunk 0's S tiles for the k-blocks just produced
                for kt in range(kc0 * 4, kc0 * 4 + 4):
                    emit_s_tile(kt, qt_cur[1], pt_cur)

            # ---- late weights for the O path ----
            wo_b = []
            for cc in range(CCH):
                t = cst.tile([128, C], BF16, name=f"wo{cc}", tag=f"wo{cc}")
                nc.sync.dma_start(out=t[:], in_=wo_d[cc * 128:(cc + 1) * 128, :])
                wo_b.append(t)
            bo_bc = cst.tile([128, C], F32)
            nc.sync.dma_start(out=bo_bc[:], in_=bo2_d[:].unsqueeze(0).broadcast_to([128, C]))

            # ---- per q-chunk: (next Q), rowsum, A fused with next S, O ----
            for qc in range(NQC):
                q0 = qc * QC
                qt_f32, qt_f8 = qt_cur
                pt_pair = pt_cur
                if qc + 1 < NQC:
                    qt_cur = emit_qproj(qc + 1)
                    pt_cur = alloc_pt()
                # rowsum via ones-matmul over partitions, then reciprocal
                rs = ps.tile([128, QC], F32, name="rs", tag="pR", bufs=2)
                for j in range(NKP):
                    nc.tensor.matmul(rs[:], lhsT=ones_f8[:], rhs=pt_pair[j][:],
                                     start=(j == 0), stop=(j == NKP - 1),
                                     perf_mode=DR)
                recip = sb.tile([128, QC], F32, name="recip", tag="recip", bufs=2)
                rscr = sb.tile([128, QC], F32, name="rscr", tag="rscr", bufs=2)
                nc.vector.reciprocal_approx_accurate(out=recip[:], in_=rs[:],
                                                     scratch=rscr[:])
                # A^T groups fused with next chunk's S/exp stream
                qa_bf = []
                for dd in range(CCH):
                    pp = ps.tile([128, QC], F32, name="aps", tag="pB", bufs=3)
                    for j in range(NKP):
                        nc.tensor.matmul(pp[:],
                                         lhsT=v_pair[j][:, :, dd * 128:(dd + 1) * 128],
                                         rhs=pt_pair[j][:],
                                         start=(j == 0), stop=(j == NKP - 1),
                                         perf_mode=DR)
                    at = sb.tile([128, QC], F32, name="at", tag="at", bufs=2)
                    nc.vector.tensor_mul(out=at[:], in0=pp[:], in1=recip[:])
                    t = sb.tile([128, QC], BF16, name=f"qa{dd}", tag=f"qa{dd}", bufs=2)
                    nc.vector.tensor_add(out=t[:], in0=at[:], in1=qt_f32[dd][:])
                    qa_bf.append(t)
                    if qc + 1 < NQC:
                        for kt in range(dd * 8, dd * 8 + 8):
                            emit_s_tile(kt, qt_cur[1], pt_cur)
                # O = (Q + A) @ Wo^T + bo2
                for rb in range(QC // 128):
                    pp = ps.tile([128, C], F32, name="ops", tag="pB", bufs=3)
                    for cc in range(CCH):
                        nc.tensor.matmul(pp[:], lhsT=qa_bf[cc][:, rb * 128:(rb + 1) * 128],
                                         rhs=wo_b[cc][:],
                                         start=(cc == 0), stop=(cc == CCH - 1))
                    ot = sb.tile([128, C], F32, name="ot", tag="ot", bufs=3)
                    nc.vector.tensor_add(out=ot[:], in0=pp[:], in1=bo_bc[:])
                    nc.sync.dma_start(out=out_d[q0 + rb * 128:q0 + (rb + 1) * 128, :],
                                      in_=ot[:])

    nc.compile()
    return nc


def get_built():
    global _BUILT
    if _BUILT is None:
        _BUILT = build()
    return _BUILT


def _pack_pairs(a_t):
    """[512, ...] row-major -> [2, 128, 2, ...] pair layout -> [256, 2, ...]."""
    rest = a_t.shape[1:]
    return np.ascontiguousarray(
        a_t.reshape(2, 2, 128, *rest).transpose(0, 2, 1, 3).reshape(256, 2, *rest))


def make_in_maps(x1, x2, Wq, bq, Wk, bk, Wv, bv, Wo, bo):
    bf = ml_dtypes.bfloat16
    f8 = ml_dtypes.float8_e4m3
    wq_t = np.ascontiguousarray(Wq.T).astype(bf)
    wk_p = _pack_pairs(np.ascontiguousarray(Wk.T).astype(f8))
    wv_p = _pack_pairs(np.ascontiguousarray(Wv.T).astype(f8))
    wo_t = np.ascontiguousarray(Wo.T).astype(bf)
    bq_p = np.ascontiguousarray(np.asarray(bq, np.float32).reshape(CCH, 128).T)
    bo2 = (Wo @ bv + bo).astype(np.float32)
    x2p_b = [_pack_pairs(np.ascontiguousarray(x2[b].T).astype(f8)) for b in range(B)]
    in_maps = []
    for cid in range(NCORES):
        b, h = cid // 2, cid % 2
        x1t = np.ascontiguousarray(x1[b, h * QROWS:(h + 1) * QROWS, :].T).astype(bf)
        in_maps.append({
            "x1t": x1t,
            "x2t_p": x2p_b[b],
            "wq_t": wq_t, "wk_p": wk_p, "wv_p": wv_p, "wo_t": wo_t,
            "bq_p": bq_p, "bo2": bo2,
        })
    return in_maps


LAST_RESULT = None


def kernel(x1, x2, Wq, bq, Wk, bk, Wv, bv, Wo, bo):
    global LAST_RESULT
    nc = get_built()
    in_maps = make_in_maps(x1, x2, Wq, bq, Wk, bk, Wv, bv, Wo, bo)
    trace = bool(os.environ.get("KERNEL_TRACE"))
    res = run_bass_kernel_spmd(nc, in_maps, core_ids=list(range(NCORES)), trace=trace)
    LAST_RESULT = res
    out = np.empty((B, N1, C), dtype=np.float32)
    for cid in range(NCORES):
        b, h = cid // 2, cid % 2
        out[b, h * QROWS:(h + 1) * QROWS, :] = res.results[cid]["out"]
    return out


# revision 8
# speedup vs baseline: 1.2088x; 1.2088x over previous
"""CrossFeatureAttention TRN2 kernel (fp8 DoubleRow attention, software-pipelined).

Full inputs -> full output. Sharding: data-parallel over (batch b, half of N1)
across 8 cores; each core computes out[b, h*2048:(h+1)*2048, :].

Math (per core, x1 slice q=2048 rows, x2[b] k=4096 rows, C=512):
    Q  = x1 @ Wq^T + bq                      (bf16 matmul, kept in fp32)
    K  = x2 @ Wk^T          (bk dropped: per-q constant in scores -> softmax inv)
    V  = x2 @ Wv^T          (bv folded into bo2 = Wo bv + bo: softmax rows sum 1)
    P  = exp(Q K^T / sqrt(C))                (S^T via fp8 DR; exp -> fp8)
    rs = colsum(P^T)  (ones fp8 DR matmul)
    A^T = V^T P^T / rs                       (fp8 DR)
    out = (Q + A) @ Wo^T + bo2               (bf16, residual folded via qt reuse)

All attention-path operands are fp8e4 packed in DoubleRow pair layout
[128, 2, N] (two 128-deep contraction planes per matmul -> 2x PE rate).
Host supplies x1^T (bf16) and x2^T / weights (fp8) pre-packed in pair layout
so no on-device transposes and one DMA per tile.

Pipelining: chunk 0's S/exp tiles are interleaved into the KV loop; chunk qc's
A phase is fused with chunk qc+1's S phase so the scalar exp stream hides
under PE work. psum->fp8 copies split across scalar and vector. Startup DMAs
are issued from both HWDGE queues (sync + scalar) to halve issue latency.
"""

import os
import sys

import numpy as np

for _p in ("/root/.axon_site", "/root/.axon_site/_ro/trn_rl_repo",
           "/root/.axon_site/_ro/pypackages"):
    if _p not in sys.path and os.path.isdir(_p):
        sys.path.append(_p)

import ml_dtypes

import concourse.bacc as bacc
import concourse.mybir as mybir
import concourse.tile as tile
from concourse.bass_utils import run_bass_kernel_spmd

F32 = mybir.dt.float32
BF16 = mybir.dt.bfloat16
F8 = mybir.dt.float8e4
AF = mybir.ActivationFunctionType
DR = mybir.MatmulPerfMode.DoubleRow

B, N1, N2, C = 4, 4096, 4096, 512
NCORES = 8
QROWS = N1 * B // NCORES          # 2048 q rows per core
QC = 512                          # q-chunk (columns of S^T tiles)
NQC = QROWS // QC                 # 4 chunks
KT = N2 // 128                    # 32 k-tiles
CCH = C // 128                    # 4 contraction planes of 128
NKP = KT // 2                     # 16 k-plane pairs
SCALE = 1.0 / float(np.sqrt(C))

_BUILT = None


def build():
    nc = bacc.Bacc(None, target_bir_lowering=False, debug=False)

    x1t_d = nc.dram_tensor("x1t", [C, QROWS], BF16, kind="ExternalInput")
    x2p_d = nc.dram_tensor("x2t_p", [256, 2, N2], F8, kind="ExternalInput")
    wq_d = nc.dram_tensor("wq_t", [C, C], BF16, kind="ExternalInput")
    wk_d = nc.dram_tensor("wk_p", [256, 2, C], F8, kind="ExternalInput")
    wv_d = nc.dram_tensor("wv_p", [256, 2, C], F8, kind="ExternalInput")
    wo_d = nc.dram_tensor("wo_t", [C, C], BF16, kind="ExternalInput")
    bq_d = nc.dram_tensor("bq_p", [128, CCH], F32, kind="ExternalInput")
    bo2_d = nc.dram_tensor("bo2", [C], F32, kind="ExternalInput")
    out_d = nc.dram_tensor("out", [QROWS, C], F32, kind="ExternalOutput")

    with tile.TileContext(nc) as tc:
        with tc.tile_pool(name="cst", bufs=1) as cst, \
             tc.tile_pool(name="per", bufs=1) as per, \
             tc.tile_pool(name="sb", bufs=1) as sb, \
             tc.tile_pool(name="ps", bufs=1, space="PSUM") as ps:

            # ---- startup loads, dual-queue: scalar gets the Q-path deps ----
            wq_b = []
            for cc in range(CCH):
                t = cst.tile([128, C], BF16, name=f"wq{cc}", tag=f"wq{cc}")
                nc.scalar.dma_start(out=t[:], in_=wq_d[cc * 128:(cc + 1) * 128, :])
                wq_b.append(t)
            bq_p = cst.tile([128, CCH], F32)
            nc.scalar.dma_start(out=bq_p[:], in_=bq_d[:])
            bq_t = [bq_p[:, d:d + 1] for d in range(CCH)]

            ones_f8 = cst.tile([128, 2, 128], F8)
            nc.gpsimd.memset(ones_f8[:], 1.0)

            def load_pair_f8(dram, nm):
                ts = []
                for j in range(2):
                    t = cst.tile([128, 2, C], F8, name=f"{nm}{j}", tag=f"{nm}{j}")
                    nc.sync.dma_start(out=t[:], in_=dram[j * 128:(j + 1) * 128, :, :])
                    ts.append(t)
                return ts

            wk_pair = load_pair_f8(wk_d, "wk")
            wv_pair = load_pair_f8(wv_d, "wv")

            # ---- persistent tensors ----
            x2t_pair = [per.tile([128, 2, N2], F8, name=f"x2t{j}", tag=f"x2t{j}")
                        for j in range(2)]
            kt_pair = [per.tile([128, 2, N2], F8, name=f"ktp{j}", tag=f"ktp{j}")
                       for j in range(2)]
            v_pair = [per.tile([128, 2, C], F8, name=f"vp{j}", tag=f"vp{j}")
                      for j in range(NKP)]

            # ---- Q^T projection for one chunk (PE + vector f32 + scalar fp8) ----
            def emit_qproj(qc):
                q0 = qc * QC
                x1bt = []
                for cc in range(CCH):
                    t = sb.tile([128, QC], BF16, name=f"x1bt{cc}", tag=f"x1bt{cc}", bufs=2)
                    nc.scalar.dma_start(out=t[:],
                                        in_=x1t_d[cc * 128:(cc + 1) * 128, q0:q0 + QC])
                    x1bt.append(t)
                qt_f32 = []
                qt_f8 = [sb.tile([128, 2, QC], F8, name=f"qt8_{j}", tag=f"qt8_{j}", bufs=2)
                         for j in range(2)]
                for dd in range(CCH):
                    pp = ps.tile([128, QC], F32, name="qps", tag="pB", bufs=3)
                    for cc in range(CCH):
                        nc.tensor.matmul(pp[:], lhsT=wq_b[cc][:, dd * 128:(dd + 1) * 128],
                                         rhs=x1bt[cc][:],
                                         start=(cc == 0), stop=(cc == CCH - 1))
                    t = sb.tile([128, QC], F32, name=f"qtf{dd}", tag=f"qtf{dd}", bufs=2)
                    nc.vector.tensor_add(out=t[:], in0=pp[:],
                                         in1=bq_t[dd].broadcast_to([128, QC]))
                    qt_f32.append(t)
                    nc.scalar.activation(qt_f8[dd // 2][:, dd % 2, :], pp[:],
                                         AF.Identity, bias=bq_t[dd])
                return qt_f32, qt_f8

            def alloc_pt():
                return [sb.tile([128, 2, QC], F8, name=f"pt{j}", tag=f"pt{j}", bufs=2)
                        for j in range(NKP)]

            def emit_s_tile(kt, qt_f8, pt_pair):
                pp = ps.tile([128, QC], F32, name="sps", tag="pA", bufs=3)
                for j in range(2):
                    nc.tensor.matmul(pp[:],
                                     lhsT=kt_pair[j][:, :, kt * 128:(kt + 1) * 128],
                                     rhs=qt_f8[j][:],
                                     start=(j == 0), stop=(j == 1),
                                     perf_mode=DR)
                nc.scalar.activation(pt_pair[kt // 2][:, kt % 2, :], pp[:],
                                     AF.Exp, scale=float(SCALE))

            qt_cur = emit_qproj(0)
            pt_cur = alloc_pt()

            # ---- phase KV (+ chunk-0 S/exp interleaved) ----
            for kc0 in range(N2 // 512):
                if kc0 % 2 == 0:
                    bsl = slice(kc0 * 512, (kc0 + 2) * 512)
                    for j in range(2):
                        nc.sync.dma_start(out=x2t_pair[j][:, :, bsl],
                                          in_=x2p_d[j * 128:(j + 1) * 128, :, bsl])
                ksl = slice(kc0 * 512, (kc0 + 1) * 512)
                # K^T[dd-plane, k-block]; bk dropped (softmax-invariant)
                for dd in range(CCH):
                    pp = ps.tile([128, 512], F32, name="kps", tag="pB", bufs=3)
                    for j in range(2):
                        nc.tensor.matmul(pp[:],
                                         lhsT=wk_pair[j][:, :, dd * 128:(dd + 1) * 128],
                                         rhs=x2t_pair[j][:, :, ksl],
                                         start=(j == 0), stop=(j == 1),
                                         perf_mode=DR)
                    if dd < 2:
                        nc.scalar.copy(kt_pair[dd // 2][:, dd % 2, ksl], pp[:])
                    else:
                        nc.vector.tensor_copy(out=kt_pair[dd // 2][:, dd % 2, ksl],
                                              in_=pp[:])
                # V[k-subtile, :]; bv folded into bo2
                for kb in range(4):
                    ki = kc0 * 4 + kb
                    pp = ps.tile([128, C], F32, name="vps", tag="pB", bufs=3)
                    for j in range(2):
                        nc.tensor.matmul(pp[:],
                                         lhsT=x2t_pair[j][:, :, ki * 128:(ki + 1) * 128],
                                         rhs=wv_pair[j][:],
                                         start=(j == 0), stop=(j == 1),
                                         perf_mode=DR)
                    nc.vector.tensor_copy(out=v_pair[ki // 2][:, ki % 2, :], in_=pp[:])
                # chunk 0's S tiles for the k-blocks just produced
                for kt in range(kc0 * 4, kc0 * 4 + 4):
                    emit_s_tile(kt, qt_cur[1], pt_cur)

            # ---- late weights for the O path ----
            wo_b = []
            for cc in range(CCH):
                t = cst.tile([128, C], BF16, name=f"wo{cc}", tag=f"wo{cc}")
                nc.sync.dma_start(out=t[:], in_=wo_d[cc * 128:(cc + 1) * 128, :])
                wo_b.append(t)
            bo_bc = cst.tile([128, C], F32)
            nc.sync.dma_start(out=bo_bc[:], in_=bo2_d[:].unsqueeze(0).broadcast_to([128, C]))

            # ---- per q-chunk: (next Q), rowsum, A fused with next S, O ----
            for qc in range(NQC):
                q0 = qc * QC
                qt_f32, qt_f8 = qt_cur
                pt_pair = pt_cur
                if qc + 1 < NQC:
                    qt_cur = emit_qproj(qc + 1)
                    pt_cur = alloc_pt()
                # rowsum via ones-matmul over partitions, then reciprocal
                rs = ps.tile([128, QC], F32, name="rs", tag="pR", bufs=2)
                for j in range(NKP):
                    nc.tensor.matmul(rs[:], lhsT=ones_f8[:], rhs=pt_pair[j][:],
                                     start=(j == 0), stop=(j == NKP - 1),
                                     perf_mode=DR)
                recip = sb.tile([128, QC], F32, name="recip", tag="recip", bufs=2)
                rscr = sb.tile([128, QC], F32, name="rscr", tag="rscr", bufs=2)
                nc.vector.reciprocal_approx_accurate(out=recip[:], in_=rs[:],
                                                     scratch=rscr[:])
                # A^T groups fused with next chunk's S/exp stream
                qa_bf = []
                for dd in range(CCH):
                    pp = ps.tile([128, QC], F32, name="aps", tag="pB", bufs=3)
                    for j in range(NKP):
                        nc.tensor.matmul(pp[:],
                                         lhsT=v_pair[j][:, :, dd * 128:(dd + 1) * 128],
                                         rhs=pt_pair[j][:],
                                         start=(j == 0), stop=(j == NKP - 1),
                                         perf_mode=DR)
                    at = sb.tile([128, QC], F32, name="at", tag="at", bufs=2)
                    nc.vector.tensor_mul(out=at[:], in0=pp[:], in1=recip[:])
                    t = sb.tile([128, QC], BF16, name=f"qa{dd}", tag=f"qa{dd}", bufs=2)
                    nc.vector.tensor_add(out=t[:], in0=at[:], in1=qt_f32[dd][:])
                    qa_bf.append(t)
                    if qc + 1 < NQC:
                        for kt in range(dd * 8, dd * 8 + 8):
                            emit_s_tile(kt, qt_cur[1], pt_cur)
                # O = (Q + A) @ Wo^T + bo2
                for rb in range(QC // 128):
                    pp = ps.tile([128, C], F32, name="ops", tag="pB", bufs=3)
                    for cc in range(CCH):
                        nc.tensor.matmul(pp[:], lhsT=qa_bf[cc][:, rb * 128:(rb + 1) * 128],
                                         rhs=wo_b[cc][:],
                                         start=(cc == 0), stop=(cc == CCH - 1))
                    ot = sb.tile([128, C], F32, name="ot", tag="ot", bufs=3)
                    nc.vector.tensor_add(out=ot[:], in0=pp[:], in1=bo_bc[:])
                    nc.sync.dma_start(out=out_d[q0 + rb * 128:q0 + (rb + 1) * 128, :],
                                      in_=ot[:])

    nc.compile()
    return nc


def get_built():
    global _BUILT
    if _BUILT is None:
        _BUILT = build()
    return _BUILT


def _pack_pairs(a_t):
    """[512, ...] row-major -> [2, 128, 2, ...] pair layout -> [256, 2, ...]."""
    rest = a_t.shape[1:]
    return np.ascontiguousarray(
        a_t.reshape(2, 2, 128, *rest).transpose(0, 2, 1, 3).reshape(256, 2, *rest))


def make_in_maps(x1, x2, Wq, bq, Wk, bk, Wv, bv, Wo, bo):
    bf = ml_dtypes.bfloat16
    f8 = ml_dtypes.float8_e4m3
    wq_t = np.ascontiguousarray(Wq.T).astype(bf)
    wk_p = _pack_pairs(np.ascontiguousarray(Wk.T).astype(f8))
    wv_p = _pack_pairs(np.ascontiguousarray(Wv.T).astype(f8))
    wo_t = np.ascontiguousarray(Wo.T).astype(bf)
    bq_p = np.ascontiguousarray(np.asarray(bq, np.float32).reshape(CCH, 128).T)
    bo2 = (Wo @ bv + bo).astype(np.float32)
    x2p_b = [_pack_pairs(np.ascontiguousarray(x2[b].T).astype(f8)) for b in range(B)]
    in_maps = []
    for cid in range(NCORES):
        b, h = cid // 2, cid % 2
        x1t = np.ascontiguousarray(x1[b, h * QROWS:(h + 1) * QROWS, :].T).astype(bf)
        in_maps.append({
            "x1t": x1t,
            "x2t_p": x2p_b[b],
            "wq_t": wq_t, "wk_p": wk_p, "wv_p": wv_p, "wo_t": wo_t,
            "bq_p": bq_p, "bo2": bo2,
        })
    return in_maps


LAST_RESULT = None


def kernel(x1, x2, Wq, bq, Wk, bk, Wv, bv, Wo, bo):
    global LAST_RESULT
    nc = get_built()
    in_maps = make_in_maps(x1, x2, Wq, bq, Wk, bk, Wv, bv, Wo, bo)
    trace = bool(os.environ.get("KERNEL_TRACE"))
    res = run_bass_kernel_spmd(nc, in_maps, core_ids=list(range(NCORES)), trace=trace)
    LAST_RESULT = res
    out = np.empty((B, N1, C), dtype=np.float32)
    for cid in range(NCORES):
        b, h = cid // 2, cid % 2
        out[b, h * QROWS:(h + 1) * QROWS, :] = res.results[cid]["out"]
    return out


# revision 11
# speedup vs baseline: 1.2197x; 1.0090x over previous
"""CrossFeatureAttention TRN2 kernel (fp8 DoubleRow attention, software-pipelined).

Full inputs -> full output. Sharding: data-parallel over (batch b, half of N1)
across 8 cores; each core computes out[b, h*2048:(h+1)*2048, :].

Math (per core, x1 slice q=2048 rows, x2[b] k=4096 rows, C=512):
    Q  = x1 @ Wq^T + bq                      (bf16 matmul, kept in fp32)
    K  = x2 @ Wk^T          (bk dropped: per-q constant in scores -> softmax inv)
    V  = x2 @ Wv^T          (bv folded into bo2 = Wo bv + bo: softmax rows sum 1)
    P  = exp(Q K^T / sqrt(C))                (S^T via fp8 DR; exp -> fp8)
    rs = colsum(P^T)  (ones fp8 DR matmul)
    A^T = V^T P^T / rs                       (fp8 DR)
    out = (Q + A) @ Wo^T + bo2               (bf16, residual folded via qt reuse)

All attention-path operands are fp8e4 packed in DoubleRow pair layout
[128, 2, N] (two 128-deep contraction planes per matmul -> 2x PE rate).
Host supplies x1^T (bf16) and x2^T / weights (fp8) pre-packed in pair layout
so no on-device transposes and one DMA per tile.

Pipelining: chunk 0's S/exp tiles are interleaved into the KV loop; chunk qc's
A phase is fused with chunk qc+1's S phase so the scalar exp stream hides
under PE work. psum->fp8 copies split across scalar and vector. Startup DMAs
are issued from both HWDGE queues (sync + scalar) to halve issue latency.
"""

import os
import sys

import numpy as np

for _p in ("/root/.axon_site", "/root/.axon_site/_ro/trn_rl_repo",
           "/root/.axon_site/_ro/pypackages"):
    if _p not in sys.path and os.path.isdir(_p):
        sys.path.append(_p)

import ml_dtypes

import concourse.bacc as bacc
import concourse.mybir as mybir
import concourse.tile as tile
from concourse.bass_utils import run_bass_kernel_spmd

F32 = mybir.dt.float32
BF16 = mybir.dt.bfloat16
F8 = mybir.dt.float8e4
AF = mybir.ActivationFunctionType
DR = mybir.MatmulPerfMode.DoubleRow

B, N1, N2, C = 4, 4096, 4096, 512
NCORES = 8
QROWS = N1 * B // NCORES          # 2048 q rows per core
QC = 512                          # q-chunk (columns of S^T tiles)
NQC = QROWS // QC                 # 4 chunks
KT = N2 // 128                    # 32 k-tiles
CCH = C // 128                    # 4 contraction planes of 128
NKP = KT // 2                     # 16 k-plane pairs
SCALE = 1.0 / float(np.sqrt(C))

_BUILT = None


def build():
    nc = bacc.Bacc(None, target_bir_lowering=False, debug=False)

    x1t_d = nc.dram_tensor("x1t", [C, QROWS], BF16, kind="ExternalInput")
    x2p_d = nc.dram_tensor("x2t_p", [256, 2, N2], F8, kind="ExternalInput")
    wq_d = nc.dram_tensor("wq_t", [C, C], BF16, kind="ExternalInput")
    wk_d = nc.dram_tensor("wk_p", [256, 2, C], F8, kind="ExternalInput")
    wv_d = nc.dram_tensor("wv_p", [256, 2, C], F8, kind="ExternalInput")
    wo_d = nc.dram_tensor("wo_t", [C, C], BF16, kind="ExternalInput")
    bq_d = nc.dram_tensor("bq_p", [128, CCH], F32, kind="ExternalInput")
    bo2_d = nc.dram_tensor("bo2", [C], F32, kind="ExternalInput")
    out_d = nc.dram_tensor("out", [QROWS, C], F32, kind="ExternalOutput")

    with tile.TileContext(nc) as tc:
        with tc.tile_pool(name="cst", bufs=1) as cst, \
             tc.tile_pool(name="per", bufs=1) as per, \
             tc.tile_pool(name="sb", bufs=1) as sb, \
             tc.tile_pool(name="ps", bufs=1, space="PSUM") as ps:

            # ---- startup loads, dual-queue: scalar gets the Q-path deps,
            # interleaved so each Q-proj matmul's operand pair lands together.
            x1bt0 = []
            wq_b = []
            for cc in range(CCH):
                t = sb.tile([128, QC], BF16, name=f"x1bt{cc}", tag=f"x1bt{cc}", bufs=2)
                nc.scalar.dma_start(out=t[:], in_=x1t_d[cc * 128:(cc + 1) * 128, 0:QC])
                x1bt0.append(t)
                t = cst.tile([128, C], BF16, name=f"wq{cc}", tag=f"wq{cc}")
                nc.scalar.dma_start(out=t[:], in_=wq_d[cc * 128:(cc + 1) * 128, :])
                wq_b.append(t)
            bq_p = cst.tile([128, CCH], F32)
            nc.scalar.dma_start(out=bq_p[:], in_=bq_d[:])
            bq_t = [bq_p[:, d:d + 1] for d in range(CCH)]

            ones_f8 = cst.tile([128, 2, 128], F8)
            nc.gpsimd.memset(ones_f8[:], 1.0)

            def load_pair_f8(dram, nm):
                ts = []
                for j in range(2):
                    t = cst.tile([128, 2, C], F8, name=f"{nm}{j}", tag=f"{nm}{j}")
                    nc.sync.dma_start(out=t[:], in_=dram[j * 128:(j + 1) * 128, :, :])
                    ts.append(t)
                return ts

            wk_pair = load_pair_f8(wk_d, "wk")

            # ---- persistent tensors ----
            x2t_pair = [per.tile([128, 2, N2], F8, name=f"x2t{j}", tag=f"x2t{j}")
                        for j in range(2)]
            kt_pair = [per.tile([128, 2, N2], F8, name=f"ktp{j}", tag=f"ktp{j}")
                       for j in range(2)]
            v_pair = [per.tile([128, 2, C], F8, name=f"vp{j}", tag=f"vp{j}")
                      for j in range(NKP)]

            def dma_x2t_block(blk):
                bsl = slice(blk * 1024, (blk + 1) * 1024)
                for j in range(2):
                    nc.sync.dma_start(out=x2t_pair[j][:, :, bsl],
                                      in_=x2p_d[j * 128:(j + 1) * 128, :, bsl])

            dma_x2t_block(0)
            wv_pair = load_pair_f8(wv_d, "wv")

            # ---- Q^T projection for one chunk (PE + vector f32 + scalar fp8) ----
            def emit_qproj(qc, x1bt=None):
                q0 = qc * QC
                if x1bt is None:
                    x1bt = []
                    for cc in range(CCH):
                        t = sb.tile([128, QC], BF16, name=f"x1bt{cc}",
                                    tag=f"x1bt{cc}", bufs=2)
                        nc.scalar.dma_start(
                            out=t[:], in_=x1t_d[cc * 128:(cc + 1) * 128, q0:q0 + QC])
                        x1bt.append(t)
                qt_f32 = []
                qt_f8 = [sb.tile([128, 2, QC], F8, name=f"qt8_{j}", tag=f"qt8_{j}", bufs=2)
                         for j in range(2)]
                for dd in range(CCH):
                    pp = ps.tile([128, QC], F32, name="qps", tag="pB", bufs=3)
                    for cc in range(CCH):
                        nc.tensor.matmul(pp[:], lhsT=wq_b[cc][:, dd * 128:(dd + 1) * 128],
                                         rhs=x1bt[cc][:],
                                         start=(cc == 0), stop=(cc == CCH - 1))
                    t = sb.tile([128, QC], F32, name=f"qtf{dd}", tag=f"qtf{dd}", bufs=2)
                    nc.vector.tensor_add(out=t[:], in0=pp[:],
                                         in1=bq_t[dd].broadcast_to([128, QC]))
                    qt_f32.append(t)
                    nc.scalar.activation(qt_f8[dd // 2][:, dd % 2, :], pp[:],
                                         AF.Identity, bias=bq_t[dd])
                return qt_f32, qt_f8

            def alloc_pt():
                return [sb.tile([128, 2, QC], F8, name=f"pt{j}", tag=f"pt{j}", bufs=2)
                        for j in range(NKP)]

            def emit_s_tile(kt, qt_f8, pt_pair):
                pp = ps.tile([128, QC], F32, name="sps", tag="pA", bufs=3)
                for j in range(2):
                    nc.tensor.matmul(pp[:],
                                     lhsT=kt_pair[j][:, :, kt * 128:(kt + 1) * 128],
                                     rhs=qt_f8[j][:],
                                     start=(j == 0), stop=(j == 1),
                                     perf_mode=DR)
                nc.scalar.activation(pt_pair[kt // 2][:, kt % 2, :], pp[:],
                                     AF.Exp, scale=float(SCALE))

            qt_cur = emit_qproj(0, x1bt=x1bt0)
            pt_cur = alloc_pt()

            # ---- phase KV (+ chunk-0 S/exp interleaved) ----
            for kc0 in range(N2 // 512):
                if kc0 % 2 == 0 and kc0 + 2 < N2 // 512:
                    dma_x2t_block(kc0 // 2 + 1)
                ksl = slice(kc0 * 512, (kc0 + 1) * 512)
                # K^T[dd-plane, k-block]; bk dropped (softmax-invariant)
                for dd in range(CCH):
                    pp = ps.tile([128, 512], F32, name="kps", tag="pB", bufs=3)
                    for j in range(2):
                        nc.tensor.matmul(pp[:],
                                         lhsT=wk_pair[j][:, :, dd * 128:(dd + 1) * 128],
                                         rhs=x2t_pair[j][:, :, ksl],
                                         start=(j == 0), stop=(j == 1),
                                         perf_mode=DR)
                    if dd < 2:
                        nc.scalar.copy(kt_pair[dd // 2][:, dd % 2, ksl], pp[:])
                    else:
                        nc.vector.tensor_copy(out=kt_pair[dd // 2][:, dd % 2, ksl],
                                              in_=pp[:])
                # V[k-subtile, :]; bv folded into bo2
                for kb in range(4):
                    ki = kc0 * 4 + kb
                    pp = ps.tile([128, C], F32, name="vps", tag="pB", bufs=3)
                    for j in range(2):
                        nc.tensor.matmul(pp[:],
                                         lhsT=x2t_pair[j][:, :, ki * 128:(ki + 1) * 128],
                                         rhs=wv_pair[j][:],
                                         start=(j == 0), stop=(j == 1),
                                         perf_mode=DR)
                    nc.vector.tensor_copy(out=v_pair[ki // 2][:, ki % 2, :], in_=pp[:])
                # chunk 0's S tiles for the k-blocks just produced
                for kt in range(kc0 * 4, kc0 * 4 + 4):
                    emit_s_tile(kt, qt_cur[1], pt_cur)

            # ---- late weights for the O path ----
            wo_b = []
            for cc in range(CCH):
                t = cst.tile([128, C], BF16, name=f"wo{cc}", tag=f"wo{cc}")
                nc.sync.dma_start(out=t[:], in_=wo_d[cc * 128:(cc + 1) * 128, :])
                wo_b.append(t)
            bo_bc = cst.tile([128, C], F32)
            nc.sync.dma_start(out=bo_bc[:], in_=bo2_d[:].unsqueeze(0).broadcast_to([128, C]))

            # ---- per q-chunk: (next Q), rowsum, A fused with next S, O ----
            for qc in range(NQC):
                q0 = qc * QC
                qt_f32, qt_f8 = qt_cur
                pt_pair = pt_cur
                if qc + 1 < NQC:
                    qt_cur = emit_qproj(qc + 1)
                    pt_cur = alloc_pt()
                # rowsum via ones-matmul over partitions, then reciprocal
                rs = ps.tile([128, QC], F32, name="rs", tag="pR", bufs=2)
                for j in range(NKP):
                    nc.tensor.matmul(rs[:], lhsT=ones_f8[:], rhs=pt_pair[j][:],
                                     start=(j == 0), stop=(j == NKP - 1),
                                     perf_mode=DR)
                recip = sb.tile([128, QC], F32, name="recip", tag="recip", bufs=2)
                rscr = sb.tile([128, QC], F32, name="rscr", tag="rscr", bufs=2)
                nc.vector.reciprocal_approx_accurate(out=recip[:], in_=rs[:],
                                                     scratch=rscr[:])
                # A^T groups fused with next chunk's S/exp stream
                qa_bf = []
                for dd in range(CCH):
                    pp = ps.tile([128, QC], F32, name="aps", tag="pB", bufs=3)
                    for j in range(NKP):
                        nc.tensor.matmul(pp[:],
                                         lhsT=v_pair[j][:, :, dd * 128:(dd + 1) * 128],
                                         rhs=pt_pair[j][:],
                                         start=(j == 0), stop=(j == NKP - 1),
                                         perf_mode=DR)
                    at = sb.tile([128, QC], F32, name="at", tag="at", bufs=2)
                    nc.vector.tensor_mul(out=at[:], in0=pp[:], in1=recip[:])
                    t = sb.tile([128, QC], BF16, name=f"qa{dd}", tag=f"qa{dd}", bufs=2)
                    nc.vector.tensor_add(out=t[:], in0=at[:], in1=qt_f32[dd][:])
                    qa_bf.append(t)
                    if qc + 1 < NQC:
                        for kt in range(dd * 8, dd * 8 + 8):
                            emit_s_tile(kt, qt_cur[1], pt_cur)
                # O = (Q + A) @ Wo^T + bo2
                for rb in range(QC // 128):
                    pp = ps.tile([128, C], F32, name="ops", tag="pB", bufs=3)
                    for cc in range(CCH):
                        nc.tensor.matmul(pp[:], lhsT=qa_bf[cc][:, rb * 128:(rb + 1) * 128],
                                         rhs=wo_b[cc][:],
                                         start=(cc == 0), stop=(cc == CCH - 1))
                    ot = sb.tile([128, C], F32, name="ot", tag="ot", bufs=3)
                    nc.vector.tensor_add(out=ot[:], in0=pp[:], in1=bo_bc[:])
                    nc.sync.dma_start(out=out_d[q0 + rb * 128:q0 + (rb + 1) * 128, :],
                                      in_=ot[:])

    nc.compile()
    return nc


def get_built():
    global _BUILT
    if _BUILT is None:
        _BUILT = build()
    return _BUILT


def _pack_pairs(a_t):
    """[512, ...] row-major -> [2, 128, 2, ...] pair layout -> [256, 2, ...]."""
    rest = a_t.shape[1:]
    return np.ascontiguousarray(
        a_t.reshape(2, 2, 128, *rest).transpose(0, 2, 1, 3).reshape(256, 2, *rest))


def make_in_maps(x1, x2, Wq, bq, Wk, bk, Wv, bv, Wo, bo):
    bf = ml_dtypes.bfloat16
    f8 = ml_dtypes.float8_e4m3
    wq_t = np.ascontiguousarray(Wq.T).astype(bf)
    wk_p = _pack_pairs(np.ascontiguousarray(Wk.T).astype(f8))
    wv_p = _pack_pairs(np.ascontiguousarray(Wv.T).astype(f8))
    wo_t = np.ascontiguousarray(Wo.T).astype(bf)
    bq_p = np.ascontiguousarray(np.asarray(bq, np.float32).reshape(CCH, 128).T)
    bo2 = (Wo @ bv + bo).astype(np.float32)
    x2p_b = [_pack_pairs(np.ascontiguousarray(x2[b].T).astype(f8)) for b in range(B)]
    in_maps = []
    for cid in range(NCORES):
        b, h = cid // 2, cid % 2
        x1t = np.ascontiguousarray(x1[b, h * QROWS:(h + 1) * QROWS, :].T).astype(bf)
        in_maps.append({
            "x1t": x1t,
            "x2t_p": x2p_b[b],
            "wq_t": wq_t, "wk_p": wk_p, "wv_p": wv_p, "wo_t": wo_t,
            "bq_p": bq_p, "bo2": bo2,
        })
    return in_maps


LAST_RESULT = None


def kernel(x1, x2, Wq, bq, Wk, bk, Wv, bv, Wo, bo):
    global LAST_RESULT
    nc = get_built()
    in_maps = make_in_maps(x1, x2, Wq, bq, Wk, bk, Wv, bv, Wo, bo)
    trace = bool(os.environ.get("KERNEL_TRACE"))
    res = run_bass_kernel_spmd(nc, in_maps, core_ids=list(range(NCORES)), trace=trace)
    LAST_RESULT = res
    out = np.empty((B, N1, C), dtype=np.float32)
    for cid in range(NCORES):
        b, h = cid // 2, cid % 2
        out[b, h * QROWS:(h + 1) * QROWS, :] = res.results[cid]["out"]
    return out


# revision 12
# speedup vs baseline: 1.2241x; 1.0036x over previous
"""CrossFeatureAttention TRN2 kernel (fp8 DoubleRow attention, software-pipelined).

Full inputs -> full output. Sharding: data-parallel over (batch b, half of N1)
across 8 cores; each core computes out[b, h*2048:(h+1)*2048, :].

Math (per core, x1 slice q=2048 rows, x2[b] k=4096 rows, C=512):
    Q  = x1 @ Wq^T + bq                      (bf16 matmul, kept in fp32)
    K  = x2 @ Wk^T          (bk dropped: per-q constant in scores -> softmax inv)
    V  = x2 @ Wv^T          (bv folded into bo2 = Wo bv + bo: softmax rows sum 1)
    P  = exp(Q K^T / sqrt(C))                (S^T via fp8 DR; exp -> fp8)
    rs = colsum(P^T)  (ones fp8 DR matmul)
    A^T = V^T P^T / rs                       (fp8 DR)
    out = (Q + A) @ Wo^T + bo2               (bf16, residual folded via qt reuse)

All attention-path operands are fp8e4 packed in DoubleRow pair layout
[128, 2, N] (two 128-deep contraction planes per matmul -> 2x PE rate).
Host supplies x1^T (bf16) and x2^T / weights (fp8) pre-packed in pair layout
so no on-device transposes and one DMA per tile.

Pipelining: chunk 0's S/exp tiles are interleaved into the KV loop; chunk qc's
A phase is fused with chunk qc+1's S phase so the scalar exp stream hides
under PE work. psum->fp8 copies split across scalar and vector. Startup DMAs
are issued from both HWDGE queues (sync + scalar) to halve issue latency.
"""

import os
import sys

import numpy as np

for _p in ("/root/.axon_site", "/root/.axon_site/_ro/trn_rl_repo",
           "/root/.axon_site/_ro/pypackages"):
    if _p not in sys.path and os.path.isdir(_p):
        sys.path.append(_p)

import ml_dtypes

import concourse.bacc as bacc
import concourse.mybir as mybir
import concourse.tile as tile
from concourse.bass_utils import run_bass_kernel_spmd

F32 = mybir.dt.float32
BF16 = mybir.dt.bfloat16
F8 = mybir.dt.float8e4
AF = mybir.ActivationFunctionType
DR = mybir.MatmulPerfMode.DoubleRow

B, N1, N2, C = 4, 4096, 4096, 512
NCORES = 8
QROWS = N1 * B // NCORES          # 2048 q rows per core
QC = 512                          # q-chunk (columns of S^T tiles)
NQC = QROWS // QC                 # 4 chunks
KT = N2 // 128                    # 32 k-tiles
CCH = C // 128                    # 4 contraction planes of 128
NKP = KT // 2                     # 16 k-plane pairs
SCALE = 1.0 / float(np.sqrt(C))

_BUILT = None


def build():
    nc = bacc.Bacc(None, target_bir_lowering=False, debug=False)

    x1p_d = nc.dram_tensor("x1_p", [128, CCH, QROWS], BF16, kind="ExternalInput")
    x2p_d = nc.dram_tensor("x2t_p", [256, 2, N2], F8, kind="ExternalInput")
    wq_d = nc.dram_tensor("wq_p", [128, CCH, C], BF16, kind="ExternalInput")
    wkv_d = nc.dram_tensor("wkv_p", [128, 8, C], F8, kind="ExternalInput")
    wo_d = nc.dram_tensor("wo_p", [128, CCH, C], BF16, kind="ExternalInput")
    bq_d = nc.dram_tensor("bq_p", [128, CCH], F32, kind="ExternalInput")
    bo2_d = nc.dram_tensor("bo2", [C], F32, kind="ExternalInput")
    out_d = nc.dram_tensor("out", [QROWS, C], F32, kind="ExternalOutput")

    with tile.TileContext(nc) as tc:
        with tc.tile_pool(name="cst", bufs=1) as cst, \
             tc.tile_pool(name="per", bufs=1) as per, \
             tc.tile_pool(name="sb", bufs=1) as sb, \
             tc.tile_pool(name="ps", bufs=1, space="PSUM") as ps:

            # ---- startup loads, dual-queue: scalar gets the Q-path deps,
            # interleaved so each Q-proj matmul's operand pair lands together.
            x1c0 = sb.tile([128, CCH, QC], BF16, name="x1c", tag="x1c", bufs=2)
            nc.scalar.dma_start(out=x1c0[:], in_=x1p_d[:, :, 0:QC])
            wq_p = cst.tile([128, CCH, C], BF16)
            nc.scalar.dma_start(out=wq_p[:], in_=wq_d[:])
            bq_p = cst.tile([128, CCH], F32)
            nc.scalar.dma_start(out=bq_p[:], in_=bq_d[:])
            bq_t = [bq_p[:, d:d + 1] for d in range(CCH)]

            ones_f8 = cst.tile([128, 2, 128], F8)
            nc.gpsimd.memset(ones_f8[:], 1.0)

            wkv_p = cst.tile([128, 8, C], F8)
            nc.sync.dma_start(out=wkv_p[:], in_=wkv_d[:])
            wk_pair = [wkv_p[:, 2 * j:2 * j + 2, :] for j in range(2)]
            wv_pair = [wkv_p[:, 4 + 2 * j:4 + 2 * j + 2, :] for j in range(2)]

            # ---- persistent tensors ----
            x2t_pair = [per.tile([128, 2, N2], F8, name=f"x2t{j}", tag=f"x2t{j}")
                        for j in range(2)]
            kt_pair = [per.tile([128, 2, N2], F8, name=f"ktp{j}", tag=f"ktp{j}")
                       for j in range(2)]
            v_pair = [per.tile([128, 2, C], F8, name=f"vp{j}", tag=f"vp{j}")
                      for j in range(NKP)]

            def dma_x2t_block(blk):
                bsl = slice(blk * 1024, (blk + 1) * 1024)
                for j in range(2):
                    nc.sync.dma_start(out=x2t_pair[j][:, :, bsl],
                                      in_=x2p_d[j * 128:(j + 1) * 128, :, bsl])

            dma_x2t_block(0)

            # ---- Q^T projection for one chunk (PE + vector f32 + scalar fp8) ----
            def emit_qproj(qc, x1c=None):
                q0 = qc * QC
                if x1c is None:
                    x1c = sb.tile([128, CCH, QC], BF16, name="x1c", tag="x1c", bufs=2)
                    nc.scalar.dma_start(out=x1c[:], in_=x1p_d[:, :, q0:q0 + QC])
                qt_f32 = []
                qt_f8 = [sb.tile([128, 2, QC], F8, name=f"qt8_{j}", tag=f"qt8_{j}", bufs=2)
                         for j in range(2)]
                for dd in range(CCH):
                    pp = ps.tile([128, QC], F32, name="qps", tag="pB", bufs=3)
                    for cc in range(CCH):
                        nc.tensor.matmul(
                            pp[:], lhsT=wq_p[:, cc, dd * 128:(dd + 1) * 128],
                            rhs=x1c[:, cc, :],
                            start=(cc == 0), stop=(cc == CCH - 1))
                    t = sb.tile([128, QC], F32, name=f"qtf{dd}", tag=f"qtf{dd}", bufs=2)
                    nc.vector.tensor_add(out=t[:], in0=pp[:],
                                         in1=bq_t[dd].broadcast_to([128, QC]))
                    qt_f32.append(t)
                    nc.scalar.activation(qt_f8[dd // 2][:, dd % 2, :], pp[:],
                                         AF.Identity, bias=bq_t[dd])
                return qt_f32, qt_f8

            def alloc_pt():
                return [sb.tile([128, 2, QC], F8, name=f"pt{j}", tag=f"pt{j}", bufs=2)
                        for j in range(NKP)]

            def emit_s_tile(kt, qt_f8, pt_pair):
                pp = ps.tile([128, QC], F32, name="sps", tag="pA", bufs=3)
                for j in range(2):
                    nc.tensor.matmul(pp[:],
                                     lhsT=kt_pair[j][:, :, kt * 128:(kt + 1) * 128],
                                     rhs=qt_f8[j][:],
                                     start=(j == 0), stop=(j == 1),
                                     perf_mode=DR)
                nc.scalar.activation(pt_pair[kt // 2][:, kt % 2, :], pp[:],
                                     AF.Exp, scale=float(SCALE))

            qt_cur = emit_qproj(0, x1c=x1c0)
            pt_cur = alloc_pt()

            # ---- phase KV (+ chunk-0 S/exp interleaved) ----
            for kc0 in range(N2 // 512):
                if kc0 % 2 == 0 and kc0 + 2 < N2 // 512:
                    dma_x2t_block(kc0 // 2 + 1)
                ksl = slice(kc0 * 512, (kc0 + 1) * 512)
                # K^T[dd-plane, k-block]; bk dropped (softmax-invariant)
                for dd in range(CCH):
                    pp = ps.tile([128, 512], F32, name="kps", tag="pB", bufs=3)
                    for j in range(2):
                        nc.tensor.matmul(pp[:],
                                         lhsT=wk_pair[j][:, :, dd * 128:(dd + 1) * 128],
                                         rhs=x2t_pair[j][:, :, ksl],
                                         start=(j == 0), stop=(j == 1),
                                         perf_mode=DR)
                    if dd < 2:
                        nc.scalar.copy(kt_pair[dd // 2][:, dd % 2, ksl], pp[:])
                    else:
                        nc.vector.tensor_copy(out=kt_pair[dd // 2][:, dd % 2, ksl],
                                              in_=pp[:])
                # V[k-subtile, :]; bv folded into bo2
                for kb in range(4):
                    ki = kc0 * 4 + kb
                    pp = ps.tile([128, C], F32, name="vps", tag="pB", bufs=3)
                    for j in range(2):
                        nc.tensor.matmul(pp[:],
                                         lhsT=x2t_pair[j][:, :, ki * 128:(ki + 1) * 128],
                                         rhs=wv_pair[j][:],
                                         start=(j == 0), stop=(j == 1),
                                         perf_mode=DR)
                    nc.vector.tensor_copy(out=v_pair[ki // 2][:, ki % 2, :], in_=pp[:])
                # chunk 0's S tiles for the k-blocks just produced
                for kt in range(kc0 * 4, kc0 * 4 + 4):
                    emit_s_tile(kt, qt_cur[1], pt_cur)

            # ---- late weights for the O path ----
            wo_p = cst.tile([128, CCH, C], BF16)
            nc.sync.dma_start(out=wo_p[:], in_=wo_d[:])
            bo_bc = cst.tile([128, C], F32)
            nc.sync.dma_start(out=bo_bc[:], in_=bo2_d[:].unsqueeze(0).broadcast_to([128, C]))

            # ---- per q-chunk: (next Q), rowsum, A fused with next S, O ----
            for qc in range(NQC):
                q0 = qc * QC
                qt_f32, qt_f8 = qt_cur
                pt_pair = pt_cur
                if qc + 1 < NQC:
                    qt_cur = emit_qproj(qc + 1)
                    pt_cur = alloc_pt()
                # rowsum via ones-matmul over partitions, then reciprocal
                rs = ps.tile([128, QC], F32, name="rs", tag="pR", bufs=2)
                for j in range(NKP):
                    nc.tensor.matmul(rs[:], lhsT=ones_f8[:], rhs=pt_pair[j][:],
                                     start=(j == 0), stop=(j == NKP - 1),
                                     perf_mode=DR)
                recip = sb.tile([128, QC], F32, name="recip", tag="recip", bufs=2)
                rscr = sb.tile([128, QC], F32, name="rscr", tag="rscr", bufs=2)
                nc.vector.reciprocal_approx_accurate(out=recip[:], in_=rs[:],
                                                     scratch=rscr[:])
                # A^T groups fused with next chunk's S/exp stream
                qa_bf = []
                for dd in range(CCH):
                    pp = ps.tile([128, QC], F32, name="aps", tag="pB", bufs=3)
                    for j in range(NKP):
                        nc.tensor.matmul(pp[:],
                                         lhsT=v_pair[j][:, :, dd * 128:(dd + 1) * 128],
                                         rhs=pt_pair[j][:],
                                         start=(j == 0), stop=(j == NKP - 1),
                                         perf_mode=DR)
                    at = sb.tile([128, QC], F32, name="at", tag="at", bufs=2)
                    nc.vector.tensor_mul(out=at[:], in0=pp[:], in1=recip[:])
                    t = sb.tile([128, QC], BF16, name=f"qa{dd}", tag=f"qa{dd}", bufs=2)
                    nc.vector.tensor_add(out=t[:], in0=at[:], in1=qt_f32[dd][:])
                    qa_bf.append(t)
                    if qc + 1 < NQC:
                        for kt in range(dd * 8, dd * 8 + 8):
                            emit_s_tile(kt, qt_cur[1], pt_cur)
                # O = (Q + A) @ Wo^T + bo2
                for rb in range(QC // 128):
                    pp = ps.tile([128, C], F32, name="ops", tag="pB", bufs=3)
                    for cc in range(CCH):
                        nc.tensor.matmul(pp[:], lhsT=qa_bf[cc][:, rb * 128:(rb + 1) * 128],
                                         rhs=wo_p[:, cc, :],
                                         start=(cc == 0), stop=(cc == CCH - 1))
                    ot = sb.tile([128, C], F32, name="ot", tag="ot", bufs=3)
                    nc.vector.tensor_add(out=ot[:], in0=pp[:], in1=bo_bc[:])
                    nc.sync.dma_start(out=out_d[q0 + rb * 128:q0 + (rb + 1) * 128, :],
                                      in_=ot[:])

    nc.compile()
    return nc


def get_built():
    global _BUILT
    if _BUILT is None:
        _BUILT = build()
    return _BUILT


def _pack_pairs(a_t):
    """[512, ...] row-major -> [2, 128, 2, ...] pair layout -> [256, 2, ...]."""
    rest = a_t.shape[1:]
    return np.ascontiguousarray(
        a_t.reshape(2, 2, 128, *rest).transpose(0, 2, 1, 3).reshape(256, 2, *rest))


def _pack_cmajor(a_t):
    """[512, N] (c-major rows) -> [128, 4, N]: out[p, cc, n] = a_t[cc*128+p, n]."""
    n = a_t.shape[1]
    return np.ascontiguousarray(a_t.reshape(CCH, 128, n).transpose(1, 0, 2))


def make_in_maps(x1, x2, Wq, bq, Wk, bk, Wv, bv, Wo, bo):
    bf = ml_dtypes.bfloat16
    f8 = ml_dtypes.float8_e4m3
    wq_p = _pack_cmajor(np.ascontiguousarray(Wq.T).astype(bf))
    wk_p = _pack_pairs(np.ascontiguousarray(Wk.T).astype(f8))
    wv_p = _pack_pairs(np.ascontiguousarray(Wv.T).astype(f8))
    wkv_p = np.ascontiguousarray(np.concatenate(
        [wk_p.reshape(2, 128, 2, C).transpose(1, 0, 2, 3).reshape(128, 4, C),
         wv_p.reshape(2, 128, 2, C).transpose(1, 0, 2, 3).reshape(128, 4, C)],
        axis=1))
    wo_p = _pack_cmajor(np.ascontiguousarray(Wo.T).astype(bf))
    bq_p = np.ascontiguousarray(np.asarray(bq, np.float32).reshape(CCH, 128).T)
    bo2 = (Wo @ bv + bo).astype(np.float32)
    x2p_b = [_pack_pairs(np.ascontiguousarray(x2[b].T).astype(f8)) for b in range(B)]
    in_maps = []
    for cid in range(NCORES):
        b, h = cid // 2, cid % 2
        x1t = np.ascontiguousarray(x1[b, h * QROWS:(h + 1) * QROWS, :].T).astype(bf)
        x1_p = _pack_cmajor(x1t)
        in_maps.append({
            "x1_p": x1_p,
            "x2t_p": x2p_b[b],
            "wq_p": wq_p, "wkv_p": wkv_p, "wo_p": wo_p,
            "bq_p": bq_p, "bo2": bo2,
        })
    return in_maps


LAST_RESULT = None


def kernel(x1, x2, Wq, bq, Wk, bk, Wv, bv, Wo, bo):
    global LAST_RESULT
    nc = get_built()
    in_maps = make_in_maps(x1, x2, Wq, bq, Wk, bk, Wv, bv, Wo, bo)
    trace = bool(os.environ.get("KERNEL_TRACE"))
    res = run_bass_kernel_spmd(nc, in_maps, core_ids=list(range(NCORES)), trace=trace)
    LAST_RESULT = res
    out = np.empty((B, N1, C), dtype=np.float32)
    for cid in range(NCORES):
        b, h = cid // 2, cid % 2
        out[b, h * QROWS:(h + 1) * QROWS, :] = res.results[cid]["out"]
    return out


# revision 13
# speedup vs baseline: 1.2305x; 1.0052x over previous
"""CrossFeatureAttention TRN2 kernel (fp8 DoubleRow attention, software-pipelined).

Full inputs -> full output. Sharding: data-parallel over (batch b, half of N1)
across 8 cores; each core computes out[b, h*2048:(h+1)*2048, :].

Math (per core, x1 slice q=2048 rows, x2[b] k=4096 rows, C=512):
    Q  = x1 @ Wq^T + bq                      (bf16 matmul, kept in fp32)
    K  = x2 @ Wk^T          (bk dropped: per-q constant in scores -> softmax inv)
    V  = x2 @ Wv^T          (bv folded into bo2 = Wo bv + bo: softmax rows sum 1)
    P  = exp(Q K^T / sqrt(C))                (S^T via fp8 DR; exp -> fp8)
    rs = colsum(P^T)  (ones fp8 DR matmul)
    A^T = V^T P^T / rs                       (fp8 DR)
    out = (Q + A) @ Wo^T + bo2               (bf16, residual folded via qt reuse)

All attention-path operands are fp8e4 packed in DoubleRow pair layout
[128, 2, N] (two 128-deep contraction planes per matmul -> 2x PE rate).
Host supplies x1^T (bf16) and x2^T / weights (fp8) pre-packed in pair layout
so no on-device transposes and one DMA per tile.

Pipelining: chunk 0's S/exp tiles are interleaved into the KV loop; chunk qc's
A phase is fused with chunk qc+1's S phase so the scalar exp stream hides
under PE work. psum->fp8 copies split across scalar and vector. Startup DMAs
are issued from both HWDGE queues (sync + scalar) to halve issue latency.
"""

import os
import sys

import numpy as np

for _p in ("/root/.axon_site", "/root/.axon_site/_ro/trn_rl_repo",
           "/root/.axon_site/_ro/pypackages"):
    if _p not in sys.path and os.path.isdir(_p):
        sys.path.append(_p)

import ml_dtypes

import concourse.bacc as bacc
import concourse.mybir as mybir
import concourse.tile as tile
from concourse.bass_utils import run_bass_kernel_spmd

F32 = mybir.dt.float32
BF16 = mybir.dt.bfloat16
F8 = mybir.dt.float8e4
AF = mybir.ActivationFunctionType
DR = mybir.MatmulPerfMode.DoubleRow

B, N1, N2, C = 4, 4096, 4096, 512
NCORES = 8
QROWS = N1 * B // NCORES          # 2048 q rows per core
QC = 512                          # q-chunk (columns of S^T tiles)
NQC = QROWS // QC                 # 4 chunks
KT = N2 // 128                    # 32 k-tiles
CCH = C // 128                    # 4 contraction planes of 128
NKP = KT // 2                     # 16 k-plane pairs
SCALE = 1.0 / float(np.sqrt(C))

_BUILT = None


def build():
    nc = bacc.Bacc(None, target_bir_lowering=False, debug=False)

    x1p_d = nc.dram_tensor("x1_p", [NQC * 128, CCH, QC], BF16, kind="ExternalInput")
    x2p_d = nc.dram_tensor("x2t_p", [(N2 // 1024) * 256, 2, 1024], F8, kind="ExternalInput")
    wq_d = nc.dram_tensor("wq_p", [128, CCH, C], BF16, kind="ExternalInput")
    wkv_d = nc.dram_tensor("wkv_p", [128, 8, C], F8, kind="ExternalInput")
    wo_d = nc.dram_tensor("wo_p", [128, CCH, C], BF16, kind="ExternalInput")
    bq_d = nc.dram_tensor("bq_p", [128, CCH], F32, kind="ExternalInput")
    bo2_d = nc.dram_tensor("bo2", [C], F32, kind="ExternalInput")
    out_d = nc.dram_tensor("out", [QROWS, C], F32, kind="ExternalOutput")

    with tile.TileContext(nc) as tc:
        with tc.tile_pool(name="cst", bufs=1) as cst, \
             tc.tile_pool(name="per", bufs=1) as per, \
             tc.tile_pool(name="sb", bufs=1) as sb, \
             tc.tile_pool(name="ps", bufs=1, space="PSUM") as ps:

            # ---- startup loads, dual-queue: scalar gets the Q-path deps,
            # interleaved so each Q-proj matmul's operand pair lands together.
            x1c0 = sb.tile([128, CCH, QC], BF16, name="x1c", tag="x1c", bufs=2)
            nc.scalar.dma_start(out=x1c0[:], in_=x1p_d[0:128, :, :])
            wq_p = cst.tile([128, CCH, C], BF16)
            nc.scalar.dma_start(out=wq_p[:], in_=wq_d[:])
            bq_p = cst.tile([128, CCH], F32)
            nc.scalar.dma_start(out=bq_p[:], in_=bq_d[:])
            bq_t = [bq_p[:, d:d + 1] for d in range(CCH)]

            ones_f8 = cst.tile([128, 2, 128], F8)
            nc.gpsimd.memset(ones_f8[:], 1.0)

            wkv_p = cst.tile([128, 8, C], F8)
            nc.sync.dma_start(out=wkv_p[:], in_=wkv_d[:])
            wk_pair = [wkv_p[:, 2 * j:2 * j + 2, :] for j in range(2)]
            wv_pair = [wkv_p[:, 4 + 2 * j:4 + 2 * j + 2, :] for j in range(2)]

            # ---- persistent tensors ----
            x2t_pair = [per.tile([128, 2, N2], F8, name=f"x2t{j}", tag=f"x2t{j}")
                        for j in range(2)]
            kt_pair = [per.tile([128, 2, N2], F8, name=f"ktp{j}", tag=f"ktp{j}")
                       for j in range(2)]
            v_pair = [per.tile([128, 2, C], F8, name=f"vp{j}", tag=f"vp{j}")
                      for j in range(NKP)]

            def dma_x2t_block(blk):
                bsl = slice(blk * 1024, (blk + 1) * 1024)
                for j in range(2):
                    nc.sync.dma_start(
                        out=x2t_pair[j][:, :, bsl],
                        in_=x2p_d[blk * 256 + j * 128:blk * 256 + (j + 1) * 128, :, :])

            dma_x2t_block(0)

            # ---- Q^T projection for one chunk (PE + vector f32 + scalar fp8) ----
            def emit_qproj(qc, x1c=None):
                q0 = qc * QC
                if x1c is None:
                    x1c = sb.tile([128, CCH, QC], BF16, name="x1c", tag="x1c", bufs=2)
                    nc.scalar.dma_start(out=x1c[:],
                                        in_=x1p_d[qc * 128:(qc + 1) * 128, :, :])
                qt_f32 = []
                qt_f8 = [sb.tile([128, 2, QC], F8, name=f"qt8_{j}", tag=f"qt8_{j}", bufs=2)
                         for j in range(2)]
                for dd in range(CCH):
                    pp = ps.tile([128, QC], F32, name="qps", tag="pB", bufs=3)
                    for cc in range(CCH):
                        nc.tensor.matmul(
                            pp[:], lhsT=wq_p[:, cc, dd * 128:(dd + 1) * 128],
                            rhs=x1c[:, cc, :],
                            start=(cc == 0), stop=(cc == CCH - 1))
                    t = sb.tile([128, QC], F32, name=f"qtf{dd}", tag=f"qtf{dd}", bufs=2)
                    nc.vector.tensor_add(out=t[:], in0=pp[:],
                                         in1=bq_t[dd].broadcast_to([128, QC]))
                    qt_f32.append(t)
                    nc.scalar.activation(qt_f8[dd // 2][:, dd % 2, :], pp[:],
                                         AF.Identity, bias=bq_t[dd])
                return qt_f32, qt_f8

            def alloc_pt():
                return [sb.tile([128, 2, QC], F8, name=f"pt{j}", tag=f"pt{j}", bufs=2)
                        for j in range(NKP)]

            def emit_s_tile(kt, qt_f8, pt_pair):
                pp = ps.tile([128, QC], F32, name="sps", tag="pA", bufs=3)
                for j in range(2):
                    nc.tensor.matmul(pp[:],
                                     lhsT=kt_pair[j][:, :, kt * 128:(kt + 1) * 128],
                                     rhs=qt_f8[j][:],
                                     start=(j == 0), stop=(j == 1),
                                     perf_mode=DR)
                nc.scalar.activation(pt_pair[kt // 2][:, kt % 2, :], pp[:],
                                     AF.Exp, scale=float(SCALE))

            qt_cur = emit_qproj(0, x1c=x1c0)
            pt_cur = alloc_pt()

            # ---- phase KV (+ chunk-0 S/exp interleaved) ----
            for kc0 in range(N2 // 512):
                if kc0 % 2 == 0 and kc0 + 2 < N2 // 512:
                    dma_x2t_block(kc0 // 2 + 1)
                ksl = slice(kc0 * 512, (kc0 + 1) * 512)
                # K^T[dd-plane, k-block]; bk dropped (softmax-invariant)
                for dd in range(CCH):
                    pp = ps.tile([128, 512], F32, name="kps", tag="pB", bufs=3)
                    for j in range(2):
                        nc.tensor.matmul(pp[:],
                                         lhsT=wk_pair[j][:, :, dd * 128:(dd + 1) * 128],
                                         rhs=x2t_pair[j][:, :, ksl],
                                         start=(j == 0), stop=(j == 1),
                                         perf_mode=DR)
                    if dd < 2:
                        nc.scalar.copy(kt_pair[dd // 2][:, dd % 2, ksl], pp[:])
                    else:
                        nc.vector.tensor_copy(out=kt_pair[dd // 2][:, dd % 2, ksl],
                                              in_=pp[:])
                # V[k-subtile, :]; bv folded into bo2
                for kb in range(4):
                    ki = kc0 * 4 + kb
                    pp = ps.tile([128, C], F32, name="vps", tag="pB", bufs=3)
                    for j in range(2):
                        nc.tensor.matmul(pp[:],
                                         lhsT=x2t_pair[j][:, :, ki * 128:(ki + 1) * 128],
                                         rhs=wv_pair[j][:],
                                         start=(j == 0), stop=(j == 1),
                                         perf_mode=DR)
                    nc.vector.tensor_copy(out=v_pair[ki // 2][:, ki % 2, :], in_=pp[:])
                # chunk 0's S tiles for the k-blocks just produced
                for kt in range(kc0 * 4, kc0 * 4 + 4):
                    emit_s_tile(kt, qt_cur[1], pt_cur)

            # ---- late weights for the O path ----
            wo_p = cst.tile([128, CCH, C], BF16)
            nc.sync.dma_start(out=wo_p[:], in_=wo_d[:])
            bo_bc = cst.tile([128, C], F32)
            nc.sync.dma_start(out=bo_bc[:], in_=bo2_d[:].unsqueeze(0).broadcast_to([128, C]))

            # ---- per q-chunk: (next Q), rowsum, A fused with next S, O ----
            for qc in range(NQC):
                q0 = qc * QC
                qt_f32, qt_f8 = qt_cur
                pt_pair = pt_cur
                if qc + 1 < NQC:
                    qt_cur = emit_qproj(qc + 1)
                    pt_cur = alloc_pt()
                # rowsum via ones-matmul over partitions, then reciprocal
                rs = ps.tile([128, QC], F32, name="rs", tag="pR", bufs=2)
                for j in range(NKP):
                    nc.tensor.matmul(rs[:], lhsT=ones_f8[:], rhs=pt_pair[j][:],
                                     start=(j == 0), stop=(j == NKP - 1),
                                     perf_mode=DR)
                recip = sb.tile([128, QC], F32, name="recip", tag="recip", bufs=2)
                rscr = sb.tile([128, QC], F32, name="rscr", tag="rscr", bufs=2)
                nc.vector.reciprocal_approx_accurate(out=recip[:], in_=rs[:],
                                                     scratch=rscr[:])
                # A^T groups fused with next chunk's S/exp stream
                qa_bf = []
                for dd in range(CCH):
                    pp = ps.tile([128, QC], F32, name="aps", tag="pB", bufs=3)
                    for j in range(NKP):
                        nc.tensor.matmul(pp[:],
                                         lhsT=v_pair[j][:, :, dd * 128:(dd + 1) * 128],
                                         rhs=pt_pair[j][:],
                                         start=(j == 0), stop=(j == NKP - 1),
                                         perf_mode=DR)
                    at = sb.tile([128, QC], F32, name="at", tag="at", bufs=2)
                    nc.vector.tensor_mul(out=at[:], in0=pp[:], in1=recip[:])
                    t = sb.tile([128, QC], BF16, name=f"qa{dd}", tag=f"qa{dd}", bufs=2)
                    nc.vector.tensor_add(out=t[:], in0=at[:], in1=qt_f32[dd][:])
                    qa_bf.append(t)
                    if qc + 1 < NQC:
                        for kt in range(dd * 8, dd * 8 + 8):
                            emit_s_tile(kt, qt_cur[1], pt_cur)
                # O = (Q + A) @ Wo^T + bo2
                for rb in range(QC // 128):
                    pp = ps.tile([128, C], F32, name="ops", tag="pB", bufs=3)
                    for cc in range(CCH):
                        nc.tensor.matmul(pp[:], lhsT=qa_bf[cc][:, rb * 128:(rb + 1) * 128],
                                         rhs=wo_p[:, cc, :],
                                         start=(cc == 0), stop=(cc == CCH - 1))
                    ot = sb.tile([128, C], F32, name="ot", tag="ot", bufs=3)
                    nc.vector.tensor_add(out=ot[:], in0=pp[:], in1=bo_bc[:])
                    nc.sync.dma_start(out=out_d[q0 + rb * 128:q0 + (rb + 1) * 128, :],
                                      in_=ot[:])

    nc.compile()
    return nc


def get_built():
    global _BUILT
    if _BUILT is None:
        _BUILT = build()
    return _BUILT


def _pack_pairs(a_t):
    """[512, ...] row-major -> [2, 128, 2, ...] pair layout -> [256, 2, ...]."""
    rest = a_t.shape[1:]
    return np.ascontiguousarray(
        a_t.reshape(2, 2, 128, *rest).transpose(0, 2, 1, 3).reshape(256, 2, *rest))


def _pack_cmajor(a_t):
    """[512, N] (c-major rows) -> [128, 4, N]: out[p, cc, n] = a_t[cc*128+p, n]."""
    n = a_t.shape[1]
    return np.ascontiguousarray(a_t.reshape(CCH, 128, n).transpose(1, 0, 2))


def make_in_maps(x1, x2, Wq, bq, Wk, bk, Wv, bv, Wo, bo):
    bf = ml_dtypes.bfloat16
    f8 = ml_dtypes.float8_e4m3
    wq_p = _pack_cmajor(np.ascontiguousarray(Wq.T).astype(bf))
    wk_p = _pack_pairs(np.ascontiguousarray(Wk.T).astype(f8))
    wv_p = _pack_pairs(np.ascontiguousarray(Wv.T).astype(f8))
    wkv_p = np.ascontiguousarray(np.concatenate(
        [wk_p.reshape(2, 128, 2, C).transpose(1, 0, 2, 3).reshape(128, 4, C),
         wv_p.reshape(2, 128, 2, C).transpose(1, 0, 2, 3).reshape(128, 4, C)],
        axis=1))
    wo_p = _pack_cmajor(np.ascontiguousarray(Wo.T).astype(bf))
    bq_p = np.ascontiguousarray(np.asarray(bq, np.float32).reshape(CCH, 128).T)
    bo2 = (Wo @ bv + bo).astype(np.float32)
    x2p_b = [
        np.ascontiguousarray(
            _pack_pairs(np.ascontiguousarray(x2[b].T).astype(f8))
            .reshape(256, 2, N2 // 1024, 1024).transpose(2, 0, 1, 3)
            .reshape((N2 // 1024) * 256, 2, 1024))
        for b in range(B)]
    in_maps = []
    for cid in range(NCORES):
        b, h = cid // 2, cid % 2
        x1t = np.ascontiguousarray(x1[b, h * QROWS:(h + 1) * QROWS, :].T).astype(bf)
        x1_p = np.ascontiguousarray(
            x1t.reshape(CCH, 128, NQC, QC).transpose(2, 1, 0, 3)
            .reshape(NQC * 128, CCH, QC))
        in_maps.append({
            "x1_p": x1_p,
            "x2t_p": x2p_b[b],
            "wq_p": wq_p, "wkv_p": wkv_p, "wo_p": wo_p,
            "bq_p": bq_p, "bo2": bo2,
        })
    return in_maps


LAST_RESULT = None


def kernel(x1, x2, Wq, bq, Wk, bk, Wv, bv, Wo, bo):
    global LAST_RESULT
    nc = get_built()
    in_maps = make_in_maps(x1, x2, Wq, bq, Wk, bk, Wv, bv, Wo, bo)
    trace = bool(os.environ.get("KERNEL_TRACE"))
    res = run_bass_kernel_spmd(nc, in_maps, core_ids=list(range(NCORES)), trace=trace)
    LAST_RESULT = res
    out = np.empty((B, N1, C), dtype=np.float32)
    for cid in range(NCORES):
        b, h = cid // 2, cid % 2
        out[b, h * QROWS:(h + 1) * QROWS, :] = res.results[cid]["out"]
    return out


# revision 14
# speedup vs baseline: 1.2428x; 1.0100x over previous
"""CrossFeatureAttention TRN2 kernel (fp8 DoubleRow attention, software-pipelined).

Full inputs -> full output. Sharding: data-parallel over (batch b, half of N1)
across 8 cores; each core computes out[b, h*2048:(h+1)*2048, :].

Math (per core, x1 slice q=2048 rows, x2[b] k=4096 rows, C=512):
    Q  = x1 @ Wq^T + bq                      (bf16 matmul, kept in fp32)
    K  = x2 @ Wk^T          (bk dropped: per-q constant in scores -> softmax inv)
    V  = x2 @ Wv^T          (bv folded into bo2 = Wo bv + bo: softmax rows sum 1)
    P  = exp(Q K^T / sqrt(C))                (S^T via fp8 DR; exp -> fp8)
    rs = colsum(P^T)  (ones fp8 DR matmul)
    A^T = V^T P^T / rs                       (fp8 DR)
    out = (Q + A) @ Wo^T + bo2               (bf16, residual folded via qt reuse)

All attention-path operands are fp8e4 packed in DoubleRow pair layout
[128, 2, N] (two 128-deep contraction planes per matmul -> 2x PE rate).
Host supplies x1^T (bf16) and x2^T / weights (fp8) pre-packed in pair layout
so no on-device transposes and one DMA per tile.

Pipelining: chunk 0's S/exp tiles are interleaved into the KV loop; chunk qc's
A phase is fused with chunk qc+1's S phase so the scalar exp stream hides
under PE work. psum->fp8 copies split across scalar and vector. Startup DMAs
are issued from both HWDGE queues (sync + scalar) to halve issue latency.
"""

import os
import sys

import numpy as np

for _p in ("/root/.axon_site", "/root/.axon_site/_ro/trn_rl_repo",
           "/root/.axon_site/_ro/pypackages"):
    if _p not in sys.path and os.path.isdir(_p):
        sys.path.append(_p)

import ml_dtypes

import concourse.bacc as bacc
import concourse.mybir as mybir
import concourse.tile as tile
from concourse.bass_utils import run_bass_kernel_spmd

F32 = mybir.dt.float32
BF16 = mybir.dt.bfloat16
F8 = mybir.dt.float8e4
AF = mybir.ActivationFunctionType
DR = mybir.MatmulPerfMode.DoubleRow

B, N1, N2, C = 4, 4096, 4096, 512
NCORES = 8
QROWS = N1 * B // NCORES          # 2048 q rows per core
QC = 512                          # q-chunk (columns of S^T tiles)
NQC = QROWS // QC                 # 4 chunks
KT = N2 // 128                    # 32 k-tiles
CCH = C // 128                    # 4 contraction planes of 128
NKP = KT // 2                     # 16 k-plane pairs
SCALE = 1.0 / float(np.sqrt(C))

_BUILT = None


def build():
    nc = bacc.Bacc(None, target_bir_lowering=False, debug=False)

    x1p_d = nc.dram_tensor("x1_p", [NQC * 128, CCH, QC], BF16, kind="ExternalInput")
    x2p_d = nc.dram_tensor("x2t_p", [(N2 // 1024) * 256, 2, 1024], F8, kind="ExternalInput")
    wq_d = nc.dram_tensor("wq_p", [128, CCH, C], BF16, kind="ExternalInput")
    wkv_d = nc.dram_tensor("wkv_p", [128, 8, C], F8, kind="ExternalInput")
    wo_d = nc.dram_tensor("wo_p", [128, CCH, C], BF16, kind="ExternalInput")
    bq_d = nc.dram_tensor("bq_p", [128, CCH], F32, kind="ExternalInput")
    bo2_d = nc.dram_tensor("bo2", [C], F32, kind="ExternalInput")
    out_d = nc.dram_tensor("out", [QROWS, C], F32, kind="ExternalOutput")

    with tile.TileContext(nc) as tc:
        with tc.tile_pool(name="cst", bufs=1) as cst, \
             tc.tile_pool(name="per", bufs=1) as per, \
             tc.tile_pool(name="sb", bufs=1) as sb, \
             tc.tile_pool(name="ps", bufs=1, space="PSUM") as ps:

            # ---- startup loads, dual-queue: scalar gets the Q-path deps,
            # interleaved so each Q-proj matmul's operand pair lands together.
            x1c0 = sb.tile([128, CCH, QC], BF16, name="x1c", tag="x1c", bufs=2)
            nc.scalar.dma_start(out=x1c0[:], in_=x1p_d[0:128, :, :])
            wq_p = cst.tile([128, CCH, C], BF16)
            nc.sync.dma_start(out=wq_p[:], in_=wq_d[:])
            bq_p = cst.tile([128, CCH], F32)
            nc.scalar.dma_start(out=bq_p[:], in_=bq_d[:])
            bq_t = [bq_p[:, d:d + 1] for d in range(CCH)]

            ones_f8 = cst.tile([128, 2, 128], F8)
            nc.gpsimd.memset(ones_f8[:], 1.0)

            wkv_p = cst.tile([128, 8, C], F8)
            nc.sync.dma_start(out=wkv_p[:], in_=wkv_d[:])
            wk_pair = [wkv_p[:, 2 * j:2 * j + 2, :] for j in range(2)]
            wv_pair = [wkv_p[:, 4 + 2 * j:4 + 2 * j + 2, :] for j in range(2)]

            # ---- persistent tensors ----
            x2t_pair = [per.tile([128, 2, N2], F8, name=f"x2t{j}", tag=f"x2t{j}")
                        for j in range(2)]
            kt_pair = [per.tile([128, 2, N2], F8, name=f"ktp{j}", tag=f"ktp{j}")
                       for j in range(2)]
            v_pair = [per.tile([128, 2, C], F8, name=f"vp{j}", tag=f"vp{j}")
                      for j in range(NKP)]

            def dma_x2t_block(blk):
                bsl = slice(blk * 1024, (blk + 1) * 1024)
                for j in range(2):
                    nc.sync.dma_start(
                        out=x2t_pair[j][:, :, bsl],
                        in_=x2p_d[blk * 256 + j * 128:blk * 256 + (j + 1) * 128, :, :])

            dma_x2t_block(0)

            # ---- Q^T projection for one chunk (PE + vector f32 + scalar fp8) ----
            def emit_qproj(qc, x1c=None):
                q0 = qc * QC
                if x1c is None:
                    x1c = sb.tile([128, CCH, QC], BF16, name="x1c", tag="x1c", bufs=2)
                    nc.scalar.dma_start(out=x1c[:],
                                        in_=x1p_d[qc * 128:(qc + 1) * 128, :, :])
                qt_f32 = []
                qt_f8 = [sb.tile([128, 2, QC], F8, name=f"qt8_{j}", tag=f"qt8_{j}", bufs=2)
                         for j in range(2)]
                for dd in range(CCH):
                    pp = ps.tile([128, QC], F32, name="qps", tag="pB", bufs=3)
                    for cc in range(CCH):
                        nc.tensor.matmul(
                            pp[:], lhsT=wq_p[:, cc, dd * 128:(dd + 1) * 128],
                            rhs=x1c[:, cc, :],
                            start=(cc == 0), stop=(cc == CCH - 1))
                    t = sb.tile([128, QC], F32, name=f"qtf{dd}", tag=f"qtf{dd}", bufs=2)
                    nc.vector.tensor_add(out=t[:], in0=pp[:],
                                         in1=bq_t[dd].broadcast_to([128, QC]))
                    qt_f32.append(t)
                    nc.scalar.activation(qt_f8[dd // 2][:, dd % 2, :], pp[:],
                                         AF.Identity, bias=bq_t[dd])
                return qt_f32, qt_f8

            def alloc_pt():
                return [sb.tile([128, 2, QC], F8, name=f"pt{j}", tag=f"pt{j}", bufs=2)
                        for j in range(NKP)]

            def emit_s_tile(kt, qt_f8, pt_pair):
                pp = ps.tile([128, QC], F32, name="sps", tag="pA", bufs=3)
                for j in range(2):
                    nc.tensor.matmul(pp[:],
                                     lhsT=kt_pair[j][:, :, kt * 128:(kt + 1) * 128],
                                     rhs=qt_f8[j][:],
                                     start=(j == 0), stop=(j == 1),
                                     perf_mode=DR)
                nc.scalar.activation(pt_pair[kt // 2][:, kt % 2, :], pp[:],
                                     AF.Exp, scale=float(SCALE))

            qt_cur = emit_qproj(0, x1c=x1c0)
            pt_cur = alloc_pt()

            # ---- phase KV (+ chunk-0 S/exp interleaved) ----
            for kc0 in range(N2 // 512):
                if kc0 % 2 == 0 and kc0 + 2 < N2 // 512:
                    dma_x2t_block(kc0 // 2 + 1)
                ksl = slice(kc0 * 512, (kc0 + 1) * 512)
                # K^T[dd-plane, k-block]; bk dropped (softmax-invariant)
                for dd in range(CCH):
                    pp = ps.tile([128, 512], F32, name="kps", tag="pB", bufs=3)
                    for j in range(2):
                        nc.tensor.matmul(pp[:],
                                         lhsT=wk_pair[j][:, :, dd * 128:(dd + 1) * 128],
                                         rhs=x2t_pair[j][:, :, ksl],
                                         start=(j == 0), stop=(j == 1),
                                         perf_mode=DR)
                    if dd < 2:
                        nc.scalar.copy(kt_pair[dd // 2][:, dd % 2, ksl], pp[:])
                    else:
                        nc.vector.tensor_copy(out=kt_pair[dd // 2][:, dd % 2, ksl],
                                              in_=pp[:])
                # V[k-subtile, :]; bv folded into bo2
                for kb in range(4):
                    ki = kc0 * 4 + kb
                    pp = ps.tile([128, C], F32, name="vps", tag="pB", bufs=3)
                    for j in range(2):
                        nc.tensor.matmul(pp[:],
                                         lhsT=x2t_pair[j][:, :, ki * 128:(ki + 1) * 128],
                                         rhs=wv_pair[j][:],
                                         start=(j == 0), stop=(j == 1),
                                         perf_mode=DR)
                    nc.vector.tensor_copy(out=v_pair[ki // 2][:, ki % 2, :], in_=pp[:])
                # chunk 0's S tiles for the k-blocks just produced
                for kt in range(kc0 * 4, kc0 * 4 + 4):
                    emit_s_tile(kt, qt_cur[1], pt_cur)

            # ---- late weights for the O path ----
            wo_p = cst.tile([128, CCH, C], BF16)
            nc.sync.dma_start(out=wo_p[:], in_=wo_d[:])
            bo_bc = cst.tile([128, C], F32)
            nc.sync.dma_start(out=bo_bc[:], in_=bo2_d[:].unsqueeze(0).broadcast_to([128, C]))

            # ---- per q-chunk: (next Q), rowsum, A fused with next S, O ----
            for qc in range(NQC):
                q0 = qc * QC
                qt_f32, qt_f8 = qt_cur
                pt_pair = pt_cur
                if qc + 1 < NQC:
                    qt_cur = emit_qproj(qc + 1)
                    pt_cur = alloc_pt()
                # rowsum via ones-matmul over partitions, then reciprocal
                rs = ps.tile([128, QC], F32, name="rs", tag="pR", bufs=2)
                for j in range(NKP):
                    nc.tensor.matmul(rs[:], lhsT=ones_f8[:], rhs=pt_pair[j][:],
                                     start=(j == 0), stop=(j == NKP - 1),
                                     perf_mode=DR)
                recip = sb.tile([128, QC], F32, name="recip", tag="recip", bufs=2)
                rscr = sb.tile([128, QC], F32, name="rscr", tag="rscr", bufs=2)
                nc.vector.reciprocal_approx_accurate(out=recip[:], in_=rs[:],
                                                     scratch=rscr[:])
                # A^T groups fused with next chunk's S/exp stream
                qa_bf = []
                for dd in range(CCH):
                    pp = ps.tile([128, QC], F32, name="aps", tag="pB", bufs=3)
                    for j in range(NKP):
                        nc.tensor.matmul(pp[:],
                                         lhsT=v_pair[j][:, :, dd * 128:(dd + 1) * 128],
                                         rhs=pt_pair[j][:],
                                         start=(j == 0), stop=(j == NKP - 1),
                                         perf_mode=DR)
                    at = sb.tile([128, QC], F32, name="at", tag="at", bufs=2)
                    nc.vector.tensor_mul(out=at[:], in0=pp[:], in1=recip[:])
                    t = sb.tile([128, QC], BF16, name=f"qa{dd}", tag=f"qa{dd}", bufs=2)
                    nc.vector.tensor_add(out=t[:], in0=at[:], in1=qt_f32[dd][:])
                    qa_bf.append(t)
                    if qc + 1 < NQC:
                        for kt in range(dd * 8, dd * 8 + 8):
                            emit_s_tile(kt, qt_cur[1], pt_cur)
                # O = (Q + A) @ Wo^T + bo2
                for rb in range(QC // 128):
                    pp = ps.tile([128, C], F32, name="ops", tag="pB", bufs=3)
                    for cc in range(CCH):
                        nc.tensor.matmul(pp[:], lhsT=qa_bf[cc][:, rb * 128:(rb + 1) * 128],
                                         rhs=wo_p[:, cc, :],
                                         start=(cc == 0), stop=(cc == CCH - 1))
                    ot = sb.tile([128, C], F32, name="ot", tag="ot", bufs=3)
                    nc.vector.tensor_add(out=ot[:], in0=pp[:], in1=bo_bc[:])
                    nc.sync.dma_start(out=out_d[q0 + rb * 128:q0 + (rb + 1) * 128, :],
                                      in_=ot[:])

    nc.compile()
    return nc


def get_built():
    global _BUILT
    if _BUILT is None:
        _BUILT = build()
    return _BUILT


def _pack_pairs(a_t):
    """[512, ...] row-major -> [2, 128, 2, ...] pair layout -> [256, 2, ...]."""
    rest = a_t.shape[1:]
    return np.ascontiguousarray(
        a_t.reshape(2, 2, 128, *rest).transpose(0, 2, 1, 3).reshape(256, 2, *rest))


def _pack_cmajor(a_t):
    """[512, N] (c-major rows) -> [128, 4, N]: out[p, cc, n] = a_t[cc*128+p, n]."""
    n = a_t.shape[1]
    return np.ascontiguousarray(a_t.reshape(CCH, 128, n).transpose(1, 0, 2))


def make_in_maps(x1, x2, Wq, bq, Wk, bk, Wv, bv, Wo, bo):
    bf = ml_dtypes.bfloat16
    f8 = ml_dtypes.float8_e4m3
    wq_p = _pack_cmajor(np.ascontiguousarray(Wq.T).astype(bf))
    wk_p = _pack_pairs(np.ascontiguousarray(Wk.T).astype(f8))
    wv_p = _pack_pairs(np.ascontiguousarray(Wv.T).astype(f8))
    wkv_p = np.ascontiguousarray(np.concatenate(
        [wk_p.reshape(2, 128, 2, C).transpose(1, 0, 2, 3).reshape(128, 4, C),
         wv_p.reshape(2, 128, 2, C).transpose(1, 0, 2, 3).reshape(128, 4, C)],
        axis=1))
    wo_p = _pack_cmajor(np.ascontiguousarray(Wo.T).astype(bf))
    bq_p = np.ascontiguousarray(np.asarray(bq, np.float32).reshape(CCH, 128).T)
    bo2 = (Wo @ bv + bo).astype(np.float32)
    x2p_b = [
        np.ascontiguousarray(
            _pack_pairs(np.ascontiguousarray(x2[b].T).astype(f8))
            .reshape(256, 2, N2 // 1024, 1024).transpose(2, 0, 1, 3)
            .reshape((N2 // 1024) * 256, 2, 1024))
        for b in range(B)]
    in_maps = []
    for cid in range(NCORES):
        b, h = cid // 2, cid % 2
        x1t = np.ascontiguousarray(x1[b, h * QROWS:(h + 1) * QROWS, :].T).astype(bf)
        x1_p = np.ascontiguousarray(
            x1t.reshape(CCH, 128, NQC, QC).transpose(2, 1, 0, 3)
            .reshape(NQC * 128, CCH, QC))
        in_maps.append({
            "x1_p": x1_p,
            "x2t_p": x2p_b[b],
            "wq_p": wq_p, "wkv_p": wkv_p, "wo_p": wo_p,
            "bq_p": bq_p, "bo2": bo2,
        })
    return in_maps


LAST_RESULT = None


def kernel(x1, x2, Wq, bq, Wk, bk, Wv, bv, Wo, bo):
    global LAST_RESULT
    nc = get_built()
    in_maps = make_in_maps(x1, x2, Wq, bq, Wk, bk, Wv, bv, Wo, bo)
    trace = bool(os.environ.get("KERNEL_TRACE"))
    res = run_bass_kernel_spmd(nc, in_maps, core_ids=list(range(NCORES)), trace=trace)
    LAST_RESULT = res
    out = np.empty((B, N1, C), dtype=np.float32)
    for cid in range(NCORES):
        b, h = cid // 2, cid % 2
        out[b, h * QROWS:(h + 1) * QROWS, :] = res.results[cid]["out"]
    return out


# revision 15
# speedup vs baseline: 1.2430x; 1.0001x over previous
"""CrossFeatureAttention TRN2 kernel (fp8 DoubleRow attention, software-pipelined).

Full inputs -> full output. Sharding: data-parallel over (batch b, half of N1)
across 8 cores; each core computes out[b, h*2048:(h+1)*2048, :].

Math (per core, x1 slice q=2048 rows, x2[b] k=4096 rows, C=512):
    Q  = x1 @ Wq^T + bq                      (bf16 matmul, kept in fp32)
    K  = x2 @ Wk^T          (bk dropped: per-q constant in scores -> softmax inv)
    V  = x2 @ Wv^T          (bv folded into bo2 = Wo bv + bo: softmax rows sum 1)
    P  = exp(Q K^T / sqrt(C))                (S^T via fp8 DR; exp -> fp8)
    rs = colsum(P^T)  (ones fp8 DR matmul)
    A^T = V^T P^T / rs                       (fp8 DR)
    out = (Q + A) @ Wo^T + bo2               (bf16, residual folded via qt reuse)

All attention-path operands are fp8e4 packed in DoubleRow pair layout
[128, 2, N] (two 128-deep contraction planes per matmul -> 2x PE rate).
Host supplies x1^T (bf16) and x2^T / weights (fp8) pre-packed in pair layout
so no on-device transposes and one DMA per tile.

Pipelining: chunk 0's S/exp tiles are interleaved into the KV loop; chunk qc's
A phase is fused with chunk qc+1's S phase so the scalar exp stream hides
under PE work. psum->fp8 copies split across scalar and vector. Startup DMAs
are issued from both HWDGE queues (sync + scalar) to halve issue latency.
"""

import os
import sys

import numpy as np

for _p in ("/root/.axon_site", "/root/.axon_site/_ro/trn_rl_repo",
           "/root/.axon_site/_ro/pypackages"):
    if _p not in sys.path and os.path.isdir(_p):
        sys.path.append(_p)

import ml_dtypes

import concourse.bacc as bacc
import concourse.mybir as mybir
import concourse.tile as tile
from concourse.bass_utils import run_bass_kernel_spmd

F32 = mybir.dt.float32
BF16 = mybir.dt.bfloat16
F8 = mybir.dt.float8e4
AF = mybir.ActivationFunctionType
DR = mybir.MatmulPerfMode.DoubleRow

B, N1, N2, C = 4, 4096, 4096, 512
NCORES = 8
QROWS = N1 * B // NCORES          # 2048 q rows per core
QC = 512                          # q-chunk (columns of S^T tiles)
NQC = QROWS // QC                 # 4 chunks
KT = N2 // 128                    # 32 k-tiles
CCH = C // 128                    # 4 contraction planes of 128
NKP = KT // 2                     # 16 k-plane pairs
SCALE = 1.0 / float(np.sqrt(C))

_BUILT = None


def build():
    nc = bacc.Bacc(None, target_bir_lowering=False, debug=False)

    x1p_d = nc.dram_tensor("x1_p", [NQC * 128, CCH, QC], BF16, kind="ExternalInput")
    x2p_d = nc.dram_tensor("x2t_p", [(N2 // 1024) * 256, 2, 1024], F8, kind="ExternalInput")
    wq_d = nc.dram_tensor("wq_p", [128, CCH, C], BF16, kind="ExternalInput")
    wkv_d = nc.dram_tensor("wkv_p", [128, 8, C], F8, kind="ExternalInput")
    wo_d = nc.dram_tensor("wo_p", [128, CCH, C], BF16, kind="ExternalInput")
    bq_d = nc.dram_tensor("bq_p", [128, CCH], F32, kind="ExternalInput")
    bo2_d = nc.dram_tensor("bo2", [C], F32, kind="ExternalInput")
    out_d = nc.dram_tensor("out", [QROWS, C], BF16, kind="ExternalOutput")

    with tile.TileContext(nc) as tc:
        with tc.tile_pool(name="cst", bufs=1) as cst, \
             tc.tile_pool(name="per", bufs=1) as per, \
             tc.tile_pool(name="sb", bufs=1) as sb, \
             tc.tile_pool(name="ps", bufs=1, space="PSUM") as ps:

            # ---- startup loads, dual-queue: scalar gets the Q-path deps,
            # interleaved so each Q-proj matmul's operand pair lands together.
            x1c0 = sb.tile([128, CCH, QC], BF16, name="x1c", tag="x1c", bufs=2)
            wq_p = cst.tile([128, CCH, C], BF16)
            nc.scalar.dma_start(out=x1c0[:, 0:2, :], in_=x1p_d[0:128, 0:2, :])
            nc.sync.dma_start(out=wq_p[:, 0:2, :], in_=wq_d[:, 0:2, :])
            nc.scalar.dma_start(out=x1c0[:, 2:4, :], in_=x1p_d[0:128, 2:4, :])
            nc.sync.dma_start(out=wq_p[:, 2:4, :], in_=wq_d[:, 2:4, :])
            bq_p = cst.tile([128, CCH], F32)
            nc.scalar.dma_start(out=bq_p[:], in_=bq_d[:])
            bq_t = [bq_p[:, d:d + 1] for d in range(CCH)]

            ones_f8 = cst.tile([128, 2, 128], F8)
            nc.gpsimd.memset(ones_f8[:], 1.0)

            wkv_p = cst.tile([128, 8, C], F8)
            nc.sync.dma_start(out=wkv_p[:], in_=wkv_d[:])
            wk_pair = [wkv_p[:, 2 * j:2 * j + 2, :] for j in range(2)]
            wv_pair = [wkv_p[:, 4 + 2 * j:4 + 2 * j + 2, :] for j in range(2)]

            # ---- persistent tensors ----
            x2t_pair = [per.tile([128, 2, N2], F8, name=f"x2t{j}", tag=f"x2t{j}")
                        for j in range(2)]
            kt_pair = [per.tile([128, 2, N2], F8, name=f"ktp{j}", tag=f"ktp{j}")
                       for j in range(2)]
            v_pair = [per.tile([128, 2, C], F8, name=f"vp{j}", tag=f"vp{j}")
                      for j in range(NKP)]

            def dma_x2t_block(blk, split=False):
                bsl = slice(blk * 1024, (blk + 1) * 1024)
                for j in range(2):
                    eng = nc.scalar if (split and j == 1) else nc.sync
                    eng.dma_start(
                        out=x2t_pair[j][:, :, bsl],
                        in_=x2p_d[blk * 256 + j * 128:blk * 256 + (j + 1) * 128, :, :])

            dma_x2t_block(0, split=True)

            # ---- Q^T projection for one chunk (PE + vector f32 + scalar fp8) ----
            def emit_qproj(qc, x1c=None):
                q0 = qc * QC
                if x1c is None:
                    x1c = sb.tile([128, CCH, QC], BF16, name="x1c", tag="x1c", bufs=2)
                    nc.scalar.dma_start(out=x1c[:],
                                        in_=x1p_d[qc * 128:(qc + 1) * 128, :, :])
                qt_f32 = []
                qt_f8 = [sb.tile([128, 2, QC], F8, name=f"qt8_{j}", tag=f"qt8_{j}", bufs=2)
                         for j in range(2)]
                for dd in range(CCH):
                    pp = ps.tile([128, QC], F32, name="qps", tag="pB", bufs=3)
                    for cc in range(CCH):
                        nc.tensor.matmul(
                            pp[:], lhsT=wq_p[:, cc, dd * 128:(dd + 1) * 128],
                            rhs=x1c[:, cc, :],
                            start=(cc == 0), stop=(cc == CCH - 1))
                    t = sb.tile([128, QC], F32, name=f"qtf{dd}", tag=f"qtf{dd}", bufs=2)
                    nc.vector.tensor_add(out=t[:], in0=pp[:],
                                         in1=bq_t[dd].broadcast_to([128, QC]))
                    qt_f32.append(t)
                    nc.scalar.activation(qt_f8[dd // 2][:, dd % 2, :], pp[:],
                                         AF.Identity, bias=bq_t[dd])
                return qt_f32, qt_f8

            def alloc_pt():
                return [sb.tile([128, 2, QC], F8, name=f"pt{j}", tag=f"pt{j}", bufs=2)
                        for j in range(NKP)]

            def emit_s_tile(kt, qt_f8, pt_pair):
                pp = ps.tile([128, QC], F32, name="sps", tag="pA", bufs=3)
                for j in range(2):
                    nc.tensor.matmul(pp[:],
                                     lhsT=kt_pair[j][:, :, kt * 128:(kt + 1) * 128],
                                     rhs=qt_f8[j][:],
                                     start=(j == 0), stop=(j == 1),
                                     perf_mode=DR)
                nc.scalar.activation(pt_pair[kt // 2][:, kt % 2, :], pp[:],
                                     AF.Exp, scale=float(SCALE))

            qt_cur = emit_qproj(0, x1c=x1c0)
            pt_cur = alloc_pt()

            # ---- phase KV (+ chunk-0 S/exp interleaved) ----
            for kc0 in range(N2 // 512):
                if kc0 % 2 == 0 and kc0 + 2 < N2 // 512:
                    dma_x2t_block(kc0 // 2 + 1)
                ksl = slice(kc0 * 512, (kc0 + 1) * 512)
                # K^T[dd-plane, k-block]; bk dropped (softmax-invariant)
                for dd in range(CCH):
                    pp = ps.tile([128, 512], F32, name="kps", tag="pB", bufs=3)
                    for j in range(2):
                        nc.tensor.matmul(pp[:],
                                         lhsT=wk_pair[j][:, :, dd * 128:(dd + 1) * 128],
                                         rhs=x2t_pair[j][:, :, ksl],
                                         start=(j == 0), stop=(j == 1),
                                         perf_mode=DR)
                    if dd < 2:
                        nc.scalar.copy(kt_pair[dd // 2][:, dd % 2, ksl], pp[:])
                    else:
                        nc.vector.tensor_copy(out=kt_pair[dd // 2][:, dd % 2, ksl],
                                              in_=pp[:])
                # V[k-subtile, :]; bv folded into bo2
                for kb in range(4):
                    ki = kc0 * 4 + kb
                    pp = ps.tile([128, C], F32, name="vps", tag="pB", bufs=3)
                    for j in range(2):
                        nc.tensor.matmul(pp[:],
                                         lhsT=x2t_pair[j][:, :, ki * 128:(ki + 1) * 128],
                                         rhs=wv_pair[j][:],
                                         start=(j == 0), stop=(j == 1),
                                         perf_mode=DR)
                    nc.vector.tensor_copy(out=v_pair[ki // 2][:, ki % 2, :], in_=pp[:])
                # chunk 0's S tiles for the k-blocks just produced
                for kt in range(kc0 * 4, kc0 * 4 + 4):
                    emit_s_tile(kt, qt_cur[1], pt_cur)

            # ---- late weights for the O path ----
            wo_p = cst.tile([128, CCH, C], BF16)
            nc.sync.dma_start(out=wo_p[:], in_=wo_d[:])
            bo_bc = cst.tile([128, C], F32)
            nc.sync.dma_start(out=bo_bc[:], in_=bo2_d[:].unsqueeze(0).broadcast_to([128, C]))

            # ---- per q-chunk: (next Q), rowsum, A fused with next S, O ----
            for qc in range(NQC):
                q0 = qc * QC
                qt_f32, qt_f8 = qt_cur
                pt_pair = pt_cur
                if qc + 1 < NQC:
                    qt_cur = emit_qproj(qc + 1)
                    pt_cur = alloc_pt()
                # rowsum via ones-matmul over partitions, then reciprocal
                rs = ps.tile([128, QC], F32, name="rs", tag="pR", bufs=2)
                for j in range(NKP):
                    nc.tensor.matmul(rs[:], lhsT=ones_f8[:], rhs=pt_pair[j][:],
                                     start=(j == 0), stop=(j == NKP - 1),
                                     perf_mode=DR)
                recip = sb.tile([128, QC], F32, name="recip", tag="recip", bufs=2)
                rscr = sb.tile([128, QC], F32, name="rscr", tag="rscr", bufs=2)
                nc.vector.reciprocal_approx_accurate(out=recip[:], in_=rs[:],
                                                     scratch=rscr[:])
                # A^T groups fused with next chunk's S/exp stream
                qa_bf = []
                for dd in range(CCH):
                    pp = ps.tile([128, QC], F32, name="aps", tag="pB", bufs=3)
                    for j in range(NKP):
                        nc.tensor.matmul(pp[:],
                                         lhsT=v_pair[j][:, :, dd * 128:(dd + 1) * 128],
                                         rhs=pt_pair[j][:],
                                         start=(j == 0), stop=(j == NKP - 1),
                                         perf_mode=DR)
                    at = sb.tile([128, QC], F32, name="at", tag="at", bufs=2)
                    nc.vector.tensor_mul(out=at[:], in0=pp[:], in1=recip[:])
                    t = sb.tile([128, QC], BF16, name=f"qa{dd}", tag=f"qa{dd}", bufs=2)
                    nc.vector.tensor_add(out=t[:], in0=at[:], in1=qt_f32[dd][:])
                    qa_bf.append(t)
                    if qc + 1 < NQC:
                        for kt in range(dd * 8, dd * 8 + 8):
                            emit_s_tile(kt, qt_cur[1], pt_cur)
                # O = (Q + A) @ Wo^T + bo2
                for rb in range(QC // 128):
                    pp = ps.tile([128, C], F32, name="ops", tag="pB", bufs=3)
                    for cc in range(CCH):
                        nc.tensor.matmul(pp[:], lhsT=qa_bf[cc][:, rb * 128:(rb + 1) * 128],
                                         rhs=wo_p[:, cc, :],
                                         start=(cc == 0), stop=(cc == CCH - 1))
                    ot = sb.tile([128, C], BF16, name="ot", tag="ot", bufs=3)
                    nc.vector.tensor_add(out=ot[:], in0=pp[:], in1=bo_bc[:])
                    nc.sync.dma_start(out=out_d[q0 + rb * 128:q0 + (rb + 1) * 128, :],
                                      in_=ot[:])

    nc.compile()
    return nc


def get_built():
    global _BUILT
    if _BUILT is None:
        _BUILT = build()
    return _BUILT


def _pack_pairs(a_t):
    """[512, ...] row-major -> [2, 128, 2, ...] pair layout -> [256, 2, ...]."""
    rest = a_t.shape[1:]
    return np.ascontiguousarray(
        a_t.reshape(2, 2, 128, *rest).transpose(0, 2, 1, 3).reshape(256, 2, *rest))


def _pack_cmajor(a_t):
    """[512, N] (c-major rows) -> [128, 4, N]: out[p, cc, n] = a_t[cc*128+p, n]."""
    n = a_t.shape[1]
    return np.ascontiguousarray(a_t.reshape(CCH, 128, n).transpose(1, 0, 2))


def make_in_maps(x1, x2, Wq, bq, Wk, bk, Wv, bv, Wo, bo):
    bf = ml_dtypes.bfloat16
    f8 = ml_dtypes.float8_e4m3
    wq_p = _pack_cmajor(np.ascontiguousarray(Wq.T).astype(bf))
    wk_p = _pack_pairs(np.ascontiguousarray(Wk.T).astype(f8))
    wv_p = _pack_pairs(np.ascontiguousarray(Wv.T).astype(f8))
    wkv_p = np.ascontiguousarray(np.concatenate(
        [wk_p.reshape(2, 128, 2, C).transpose(1, 0, 2, 3).reshape(128, 4, C),
         wv_p.reshape(2, 128, 2, C).transpose(1, 0, 2, 3).reshape(128, 4, C)],
        axis=1))
    wo_p = _pack_cmajor(np.ascontiguousarray(Wo.T).astype(bf))
    bq_p = np.ascontiguousarray(np.asarray(bq, np.float32).reshape(CCH, 128).T)
    bo2 = (Wo @ bv + bo).astype(np.float32)
    x2p_b = [
        np.ascontiguousarray(
            _pack_pairs(np.ascontiguousarray(x2[b].T).astype(f8))
            .reshape(256, 2, N2 // 1024, 1024).transpose(2, 0, 1, 3)
            .reshape((N2 // 1024) * 256, 2, 1024))
        for b in range(B)]
    in_maps = []
    for cid in range(NCORES):
        b, h = cid // 2, cid % 2
        x1t = np.ascontiguousarray(x1[b, h * QROWS:(h + 1) * QROWS, :].T).astype(bf)
        x1_p = np.ascontiguousarray(
            x1t.reshape(CCH, 128, NQC, QC).transpose(2, 1, 0, 3)
            .reshape(NQC * 128, CCH, QC))
        in_maps.append({
            "x1_p": x1_p,
            "x2t_p": x2p_b[b],
            "wq_p": wq_p, "wkv_p": wkv_p, "wo_p": wo_p,
            "bq_p": bq_p, "bo2": bo2,
        })
    return in_maps


LAST_RESULT = None


def kernel(x1, x2, Wq, bq, Wk, bk, Wv, bv, Wo, bo):
    global LAST_RESULT
    nc = get_built()
    in_maps = make_in_maps(x1, x2, Wq, bq, Wk, bk, Wv, bv, Wo, bo)
    trace = bool(os.environ.get("KERNEL_TRACE"))
    res = run_bass_kernel_spmd(nc, in_maps, core_ids=list(range(NCORES)), trace=trace)
    LAST_RESULT = res
    out = np.empty((B, N1, C), dtype=np.float32)
    for cid in range(NCORES):
        b, h = cid // 2, cid % 2
        out[b, h * QROWS:(h + 1) * QROWS, :] = np.asarray(
            res.results[cid]["out"], dtype=np.float32)
    return out


# revision 16
# speedup vs baseline: 1.2475x; 1.0036x over previous
"""CrossFeatureAttention TRN2 kernel (fp8 DoubleRow attention, software-pipelined).

Full inputs -> full output. Sharding: data-parallel over (batch b, half of N1)
across 8 cores; each core computes out[b, h*2048:(h+1)*2048, :].

Math (per core, x1 slice q=2048 rows, x2[b] k=4096 rows, C=512):
    Q  = x1 @ Wq^T + bq                      (bf16 matmul, kept in fp32)
    K  = x2 @ Wk^T          (bk dropped: per-q constant in scores -> softmax inv)
    V  = x2 @ Wv^T          (bv folded into bo2 = Wo bv + bo: softmax rows sum 1)
    P  = exp(Q K^T / sqrt(C))                (S^T via fp8 DR; exp -> fp8)
    rs = colsum(P^T)  (ones fp8 DR matmul)
    A^T = V^T P^T / rs                       (fp8 DR)
    out = (Q + A) @ Wo^T + bo2               (bf16, residual folded via qt reuse)

All attention-path operands are fp8e4 packed in DoubleRow pair layout
[128, 2, N] (two 128-deep contraction planes per matmul -> 2x PE rate).
Host supplies x1^T (bf16) and x2^T / weights (fp8) pre-packed in pair layout
so no on-device transposes and one DMA per tile.

Pipelining: chunk 0's S/exp tiles are interleaved into the KV loop; chunk qc's
A phase is fused with chunk qc+1's S phase so the scalar exp stream hides
under PE work. psum->fp8 copies split across scalar and vector. Startup DMAs
are issued from both HWDGE queues (sync + scalar) to halve issue latency.
"""

import os
import sys

import numpy as np

for _p in ("/root/.axon_site", "/root/.axon_site/_ro/trn_rl_repo",
           "/root/.axon_site/_ro/pypackages"):
    if _p not in sys.path and os.path.isdir(_p):
        sys.path.append(_p)

import ml_dtypes

import concourse.bacc as bacc
import concourse.mybir as mybir
import concourse.tile as tile
from concourse.bass_utils import run_bass_kernel_spmd

F32 = mybir.dt.float32
BF16 = mybir.dt.bfloat16
F8 = mybir.dt.float8e4
AF = mybir.ActivationFunctionType
DR = mybir.MatmulPerfMode.DoubleRow

B, N1, N2, C = 4, 4096, 4096, 512
NCORES = 8
QROWS = N1 * B // NCORES          # 2048 q rows per core
QC = 512                          # q-chunk (columns of S^T tiles)
NQC = QROWS // QC                 # 4 chunks
KT = N2 // 128                    # 32 k-tiles
CCH = C // 128                    # 4 contraction planes of 128
NKP = KT // 2                     # 16 k-plane pairs
SCALE = 1.0 / float(np.sqrt(C))

_BUILT = None


def build():
    nc = bacc.Bacc(None, target_bir_lowering=False, debug=False)

    x1p_d = nc.dram_tensor("x1_p", [NQC * 128, CCH, QC], BF16, kind="ExternalInput")
    x2p_d = nc.dram_tensor("x2t_p", [(N2 // 1024) * 256, 2, 1024], F8, kind="ExternalInput")
    wq_d = nc.dram_tensor("wq_p", [128, CCH, C], BF16, kind="ExternalInput")
    wkv_d = nc.dram_tensor("wkv_p", [128, 8, C], F8, kind="ExternalInput")
    wo_d = nc.dram_tensor("wo_p", [128, CCH, C], BF16, kind="ExternalInput")
    bq_d = nc.dram_tensor("bq_p", [128, CCH], F32, kind="ExternalInput")
    bo2_d = nc.dram_tensor("bo2", [C], F32, kind="ExternalInput")
    out_d = nc.dram_tensor("out", [QROWS, C], BF16, kind="ExternalOutput")

    with tile.TileContext(nc) as tc:
        with tc.tile_pool(name="cst", bufs=1) as cst, \
             tc.tile_pool(name="per", bufs=1) as per, \
             tc.tile_pool(name="sb", bufs=1) as sb, \
             tc.tile_pool(name="ps", bufs=1, space="PSUM") as ps:

            # ---- startup loads, dual-queue: scalar gets the Q-path deps,
            # interleaved so each Q-proj matmul's operand pair lands together.
            x1c0 = sb.tile([128, CCH, QC], BF16, name="x1c", tag="x1c", bufs=2)
            wq_p = cst.tile([128, CCH, C], BF16)
            nc.scalar.dma_start(out=x1c0[:], in_=x1p_d[0:128, :, :])
            nc.sync.dma_start(out=wq_p[:], in_=wq_d[:])
            bq_p = cst.tile([128, CCH], F32)
            nc.scalar.dma_start(out=bq_p[:], in_=bq_d[:])
            bq_t = [bq_p[:, d:d + 1] for d in range(CCH)]

            ones_f8 = cst.tile([128, 2, 128], F8)
            nc.gpsimd.memset(ones_f8[:], 1.0)

            wkv_p = cst.tile([128, 8, C], F8)
            nc.sync.dma_start(out=wkv_p[:], in_=wkv_d[:])
            wk_pair = [wkv_p[:, 2 * j:2 * j + 2, :] for j in range(2)]
            wv_pair = [wkv_p[:, 4 + 2 * j:4 + 2 * j + 2, :] for j in range(2)]

            # ---- persistent tensors ----
            x2t_pair = [per.tile([128, 2, N2], F8, name=f"x2t{j}", tag=f"x2t{j}")
                        for j in range(2)]
            kt_pair = [per.tile([128, 2, N2], F8, name=f"ktp{j}", tag=f"ktp{j}")
                       for j in range(2)]
            v_pair = [per.tile([128, 2, C], F8, name=f"vp{j}", tag=f"vp{j}")
                      for j in range(NKP)]

            def dma_x2t_block(blk, split=False):
                bsl = slice(blk * 1024, (blk + 1) * 1024)
                for j in range(2):
                    eng = nc.scalar if (split and j == 1) else nc.sync
                    eng.dma_start(
                        out=x2t_pair[j][:, :, bsl],
                        in_=x2p_d[blk * 256 + j * 128:blk * 256 + (j + 1) * 128, :, :])

            dma_x2t_block(0, split=True)

            # ---- Q^T projection for one chunk (PE + vector f32 + scalar fp8) ----
            def emit_qproj(qc, x1c=None):
                q0 = qc * QC
                if x1c is None:
                    x1c = sb.tile([128, CCH, QC], BF16, name="x1c", tag="x1c", bufs=2)
                    nc.scalar.dma_start(out=x1c[:],
                                        in_=x1p_d[qc * 128:(qc + 1) * 128, :, :])
                qt_f32 = []
                qt_f8 = [sb.tile([128, 2, QC], F8, name=f"qt8_{j}", tag=f"qt8_{j}", bufs=2)
                         for j in range(2)]
                for dd in range(CCH):
                    pp = ps.tile([128, QC], F32, name="qps", tag="pB", bufs=3)
                    for cc in range(CCH):
                        nc.tensor.matmul(
                            pp[:], lhsT=wq_p[:, cc, dd * 128:(dd + 1) * 128],
                            rhs=x1c[:, cc, :],
                            start=(cc == 0), stop=(cc == CCH - 1))
                    t = sb.tile([128, QC], F32, name=f"qtf{dd}", tag=f"qtf{dd}", bufs=2)
                    nc.vector.tensor_add(out=t[:], in0=pp[:],
                                         in1=bq_t[dd].broadcast_to([128, QC]))
                    qt_f32.append(t)
                    nc.scalar.activation(qt_f8[dd // 2][:, dd % 2, :], pp[:],
                                         AF.Identity, bias=bq_t[dd])
                return qt_f32, qt_f8

            def alloc_pt():
                return [sb.tile([128, 2, QC], F8, name=f"pt{j}", tag=f"pt{j}", bufs=2)
                        for j in range(NKP)]

            def emit_s_tile(kt, qt_f8, pt_pair):
                pp = ps.tile([128, QC], F32, name="sps", tag="pA", bufs=3)
                for j in range(2):
                    nc.tensor.matmul(pp[:],
                                     lhsT=kt_pair[j][:, :, kt * 128:(kt + 1) * 128],
                                     rhs=qt_f8[j][:],
                                     start=(j == 0), stop=(j == 1),
                                     perf_mode=DR)
                nc.scalar.activation(pt_pair[kt // 2][:, kt % 2, :], pp[:],
                                     AF.Exp, scale=float(SCALE))

            qt_cur = emit_qproj(0, x1c=x1c0)
            pt_cur = alloc_pt()

            # ---- phase KV (+ chunk-0 S/exp interleaved) ----
            for kc0 in range(N2 // 512):
                if kc0 == 0:
                    dma_x2t_block(1)
                    dma_x2t_block(2)
                elif kc0 == 2:
                    dma_x2t_block(3)
                ksl = slice(kc0 * 512, (kc0 + 1) * 512)
                # K^T[dd-plane, k-block]; bk dropped (softmax-invariant)
                for dd in range(CCH):
                    pp = ps.tile([128, 512], F32, name="kps", tag="pB", bufs=3)
                    for j in range(2):
                        nc.tensor.matmul(pp[:],
                                         lhsT=wk_pair[j][:, :, dd * 128:(dd + 1) * 128],
                                         rhs=x2t_pair[j][:, :, ksl],
                                         start=(j == 0), stop=(j == 1),
                                         perf_mode=DR)
                    if dd < 2:
                        nc.scalar.copy(kt_pair[dd // 2][:, dd % 2, ksl], pp[:])
                    else:
                        nc.vector.tensor_copy(out=kt_pair[dd // 2][:, dd % 2, ksl],
                                              in_=pp[:])
                # V[k-subtile, :]; bv folded into bo2
                for kb in range(4):
                    ki = kc0 * 4 + kb
                    pp = ps.tile([128, C], F32, name="vps", tag="pB", bufs=3)
                    for j in range(2):
                        nc.tensor.matmul(pp[:],
                                         lhsT=x2t_pair[j][:, :, ki * 128:(ki + 1) * 128],
                                         rhs=wv_pair[j][:],
                                         start=(j == 0), stop=(j == 1),
                                         perf_mode=DR)
                    nc.vector.tensor_copy(out=v_pair[ki // 2][:, ki % 2, :], in_=pp[:])
                # chunk 0's S tiles for the k-blocks just produced
                for kt in range(kc0 * 4, kc0 * 4 + 4):
                    emit_s_tile(kt, qt_cur[1], pt_cur)

            # ---- late weights for the O path ----
            wo_p = cst.tile([128, CCH, C], BF16)
            nc.sync.dma_start(out=wo_p[:], in_=wo_d[:])
            bo_bc = cst.tile([128, C], F32)
            nc.sync.dma_start(out=bo_bc[:], in_=bo2_d[:].unsqueeze(0).broadcast_to([128, C]))

            # ---- per q-chunk: (next Q), rowsum, A fused with next S, O ----
            for qc in range(NQC):
                q0 = qc * QC
                qt_f32, qt_f8 = qt_cur
                pt_pair = pt_cur
                if qc + 1 < NQC:
                    qt_cur = emit_qproj(qc + 1)
                    pt_cur = alloc_pt()
                # rowsum via ones-matmul over partitions, then reciprocal
                rs = ps.tile([128, QC], F32, name="rs", tag="pR", bufs=2)
                for j in range(NKP):
                    nc.tensor.matmul(rs[:], lhsT=ones_f8[:], rhs=pt_pair[j][:],
                                     start=(j == 0), stop=(j == NKP - 1),
                                     perf_mode=DR)
                recip = sb.tile([128, QC], F32, name="recip", tag="recip", bufs=2)
                rscr = sb.tile([128, QC], F32, name="rscr", tag="rscr", bufs=2)
                nc.vector.reciprocal_approx_accurate(out=recip[:], in_=rs[:],
                                                     scratch=rscr[:])
                # A^T groups fused with next chunk's S/exp stream
                qa_bf = []
                for dd in range(CCH):
                    pp = ps.tile([128, QC], F32, name="aps", tag="pB", bufs=3)
                    for j in range(NKP):
                        nc.tensor.matmul(pp[:],
                                         lhsT=v_pair[j][:, :, dd * 128:(dd + 1) * 128],
                                         rhs=pt_pair[j][:],
                                         start=(j == 0), stop=(j == NKP - 1),
                                         perf_mode=DR)
                    at = sb.tile([128, QC], F32, name="at", tag="at", bufs=2)
                    nc.vector.tensor_mul(out=at[:], in0=pp[:], in1=recip[:])
                    t = sb.tile([128, QC], BF16, name=f"qa{dd}", tag=f"qa{dd}", bufs=2)
                    nc.vector.tensor_add(out=t[:], in0=at[:], in1=qt_f32[dd][:])
                    qa_bf.append(t)
                    if qc + 1 < NQC:
                        for kt in range(dd * 8, dd * 8 + 8):
                            emit_s_tile(kt, qt_cur[1], pt_cur)
                # O = (Q + A) @ Wo^T + bo2
                for rb in range(QC // 128):
                    pp = ps.tile([128, C], F32, name="ops", tag="pB", bufs=3)
                    for cc in range(CCH):
                        nc.tensor.matmul(pp[:], lhsT=qa_bf[cc][:, rb * 128:(rb + 1) * 128],
                                         rhs=wo_p[:, cc, :],
                                         start=(cc == 0), stop=(cc == CCH - 1))
                    ot = sb.tile([128, C], BF16, name="ot", tag="ot", bufs=3)
                    nc.vector.tensor_add(out=ot[:], in0=pp[:], in1=bo_bc[:])
                    nc.sync.dma_start(out=out_d[q0 + rb * 128:q0 + (rb + 1) * 128, :],
                                      in_=ot[:])

    nc.compile()
    return nc


def get_built():
    global _BUILT
    if _BUILT is None:
        _BUILT = build()
    return _BUILT


def _pack_pairs(a_t):
    """[512, ...] row-major -> [2, 128, 2, ...] pair layout -> [256, 2, ...]."""
    rest = a_t.shape[1:]
    return np.ascontiguousarray(
        a_t.reshape(2, 2, 128, *rest).transpose(0, 2, 1, 3).reshape(256, 2, *rest))


def _pack_cmajor(a_t):
    """[512, N] (c-major rows) -> [128, 4, N]: out[p, cc, n] = a_t[cc*128+p, n]."""
    n = a_t.shape[1]
    return np.ascontiguousarray(a_t.reshape(CCH, 128, n).transpose(1, 0, 2))


def make_in_maps(x1, x2, Wq, bq, Wk, bk, Wv, bv, Wo, bo):
    bf = ml_dtypes.bfloat16
    f8 = ml_dtypes.float8_e4m3
    wq_p = _pack_cmajor(np.ascontiguousarray(Wq.T).astype(bf))
    wk_p = _pack_pairs(np.ascontiguousarray(Wk.T).astype(f8))
    wv_p = _pack_pairs(np.ascontiguousarray(Wv.T).astype(f8))
    wkv_p = np.ascontiguousarray(np.concatenate(
        [wk_p.reshape(2, 128, 2, C).transpose(1, 0, 2, 3).reshape(128, 4, C),
         wv_p.reshape(2, 128, 2, C).transpose(1, 0, 2, 3).reshape(128, 4, C)],
        axis=1))
    wo_p = _pack_cmajor(np.ascontiguousarray(Wo.T).astype(bf))
    bq_p = np.ascontiguousarray(np.asarray(bq, np.float32).reshape(CCH, 128).T)
    bo2 = (Wo @ bv + bo).astype(np.float32)
    x2p_b = [
        np.ascontiguousarray(
            _pack_pairs(np.ascontiguousarray(x2[b].T).astype(f8))
            .reshape(256, 2, N2 // 1024, 1024).transpose(2, 0, 1, 3)
            .reshape((N2 // 1024) * 256, 2, 1024))
        for b in range(B)]
    in_maps = []
    for cid in range(NCORES):
        b, h = cid // 2, cid % 2
        x1t = np.ascontiguousarray(x1[b, h * QROWS:(h + 1) * QROWS, :].T).astype(bf)
        x1_p = np.ascontiguousarray(
            x1t.reshape(CCH, 128, NQC, QC).transpose(2, 1, 0, 3)
            .reshape(NQC * 128, CCH, QC))
        in_maps.append({
            "x1_p": x1_p,
            "x2t_p": x2p_b[b],
            "wq_p": wq_p, "wkv_p": wkv_p, "wo_p": wo_p,
            "bq_p": bq_p, "bo2": bo2,
        })
    return in_maps


LAST_RESULT = None


def kernel(x1, x2, Wq, bq, Wk, bk, Wv, bv, Wo, bo):
    global LAST_RESULT
    nc = get_built()
    in_maps = make_in_maps(x1, x2, Wq, bq, Wk, bk, Wv, bv, Wo, bo)
    trace = bool(os.environ.get("KERNEL_TRACE"))
    res = run_bass_kernel_spmd(nc, in_maps, core_ids=list(range(NCORES)), trace=trace)
    LAST_RESULT = res
    out = np.empty((B, N1, C), dtype=np.float32)
    for cid in range(NCORES):
        b, h = cid // 2, cid % 2
        out[b, h * QROWS:(h + 1) * QROWS, :] = np.asarray(
            res.results[cid]["out"], dtype=np.float32)
    return out
